# revision 1
# baseline (speedup 1.0000x reference)
"""Trainium2 Bass kernel for nn_DecoderVectorized (axial decoder with causal
cross-attention). Self-contained: hardcodes all shapes/sharding.

Sharding: 32 SPMD slots = 8 cores x 4 slots over the B*(T-1)=30 frames
(sorted by t so slot j has kv prefix 128*{4,8,12,15}; 2 dummy slots).
"""
import math
import sys

import numpy as np

try:
    import concourse.bass as bass
except ImportError:  # pragma: no cover
    sys.path.insert(0, "/opt/trn_rl_repo")
    import concourse.bass as bass

import concourse.bacc as bacc
import concourse.mybir as mybir
import concourse.tile as tile
from concourse import bass_utils
from concourse.masks import make_identity

F32 = mybir.dt.float32
F32R = mybir.dt.float32r
BF16 = mybir.dt.bfloat16
import ml_dtypes
NPBF = ml_dtypes.bfloat16
AF = mybir.ActivationFunctionType
OP = mybir.AluOpType

H, W, D, HEADS, QL = 16, 16, 192, 8, 256
B, T, M = 2, 16, 128
NQ = H * W          # 256 tokens per frame
DH = D // HEADS     # 24
NCORE, NSLOT = 8, 4
TMAX = [4, 8, 12, 16]
KV = [t * 128 for t in TMAX]        # 512 1024 1536 2048
CM = 32.0                           # mask bias (power of 2: bf16-exact)
SCL = 1.0 / math.sqrt(DH)
EPS = 1e-5


def _frame(f):
    """frame index f in [0,32) -> (b, t); 30/31 are dummies."""
    if f >= 30:
        return (f - 30, 15)
    return (f % 2, f // 2 + 1)


# ---------------------------------------------------------------- host prep

def _qk_colmat_s1(w, bvec, g, is_q):
    """[193,128] colmat for stage-1 qhT/khT half g. Head h data at cols
    32h+1..32h+24 (aux row 0 first); col 32h+0: e_192 (ones) if is_q."""
    m = np.zeros((193, 128), np.float32)
    for h in range(4):
        Hh = 4 * g + h
        m[0:192, 32 * h + 1:32 * h + 25] = w[:, DH * Hh:DH * Hh + DH]
        m[192, 32 * h + 1:32 * h + 25] = bvec[DH * Hh:DH * Hh + DH]
        if is_q:
            m[192, 32 * h] = 1.0
    return m


def _qk_colmat_23(w, bvec, ch, g):
    """[97,128] colmat chunk for stage-2/3 qhT/khT. Data at cols 32h+3..+26;
    aux cols 0..2 zero (DMA'd)."""
    m = np.zeros((97, 128), np.float32)
    for h in range(4):
        Hh = 4 * g + h
        m[0:96, 32 * h + 3:32 * h + 27] = w[96 * ch:96 * ch + 96, DH * Hh:DH * Hh + DH]
        m[96, 32 * h + 3:32 * h + 27] = bvec[DH * Hh:DH * Hh + DH] * 0.5
    return m


def _wv_colmat_s1(w, bvec):
    """[193,256]: head Hh data at cols 32Hh..+23, ones col at 32Hh+24."""
    m = np.zeros((193, 256), np.float32)
    for Hh in range(8):
        m[0:192, 32 * Hh:32 * Hh + 24] = w[:, DH * Hh:DH * Hh + DH]
        m[192, 32 * Hh:32 * Hh + 24] = bvec[DH * Hh:DH * Hh + DH]
        m[192, 32 * Hh + 24] = 1.0
    return m


def _wv_colmat_23(w, bvec, ch):
    m = np.zeros((97, 256), np.float32)
    for Hh in range(8):
        m[0:96, 32 * Hh:32 * Hh + 24] = w[96 * ch:96 * ch + 96, DH * Hh:DH * Hh + DH]
        m[96, 32 * Hh:32 * Hh + 24] = bvec[DH * Hh:DH * Hh + DH] * 0.5
        m[96, 32 * Hh + 24] = 0.5
    return m


def _wo_aug(w, bvec):
    """[128, 384]: head H=4g+h at partition rows 32h..32h+31, col block 192g:
    rows 0..23 = wo rows, row 24 = bo/8 (walrus needs lhsT/rhs same base)."""
    m = np.zeros((128, 2 * D), np.float32)
    for g in range(2):
        for h in range(4):
            Hh = 4 * g + h
            m[32 * h:32 * h + 24, D * g:D * g + D] = w[DH * Hh:DH * Hh + DH, :]
            m[32 * h + 24, D * g:D * g + D] = bvec / 8.0
    return m


def _aux_rows(idx, is_q):
    """[128,256] aux contraction rows for the rank-3 axial mask, pre-spread to
    partition rows 32h+0..2. k-side: [ri^2, ri, 1]; q-side: [-c, 2c rj, -c rj^2]."""
    r = idx.astype(np.float32)
    if is_q:
        rows = np.stack([np.full(NQ, -CM, np.float32), 2.0 * CM * r, -CM * r * r])
    else:
        rows = np.stack([r * r, r, np.ones(NQ, np.float32)])
    m = np.zeros((128, NQ), np.float32)
    for h in range(4):
        m[32 * h:32 * h + 3] = rows
    return m


def _host_constants(inp):
    """Shared (core-independent) device constant arrays."""
    c = {}
    g, b_ = inp["rn_g"], inp["rn_b"]

    def eff(wq, bq, scale):
        return (g[:, None] * wq * scale).astype(np.float32), \
               ((b_ @ wq + bq) * scale).astype(np.float32)

    for gg in range(2):
        c[f"cwq{gg}"] = _qk_colmat_s1(inp["c_wq"] * SCL, inp["c_bq"] * SCL, gg, True)
        c[f"cwk{gg}"] = _qk_colmat_s1(inp["c_wk"], inp["c_bk"], gg, False)
    c["cwv"] = _wv_colmat_s1(inp["c_wv"], inp["c_bv"])
    c["cwo"] = _wo_aug(inp["c_wo"], inp["c_bo"])
    tok = np.arange(NQ)
    for pre, wp, idx in (("r", "r", tok // 16), ("l", "col", tok % 16)):
        wq, bq = eff(inp[wp + "_wq"], inp[wp + "_bq"], SCL)
        wk, bk = eff(inp[wp + "_wk"], inp[wp + "_bk"], 1.0)
        wv, bv = eff(inp[wp + "_wv"], inp[wp + "_bv"], 1.0)
        for ch in range(2):
            for gg in range(2):
                c[f"{pre}wq{ch}{gg}"] = _qk_colmat_23(wq, bq, ch, gg)
                c[f"{pre}wk{ch}{gg}"] = _qk_colmat_23(wk, bk, ch, gg)
            c[f"{pre}wv{ch}"] = _wv_colmat_23(wv, bv, ch)
        c[f"{pre}wo"] = _wo_aug(inp[wp + "_wo"], inp[wp + "_bo"])
        c[f"{pre}ka"] = _aux_rows(idx, False)
        c[f"{pre}qa"] = _aux_rows(idx, True)
    w1 = (inp["ff_ln_g"][:, None] * inp["ff_w1"]).astype(np.float32)
    b1 = (inp["ff_ln_b"] @ inp["ff_w1"] + inp["ff_b1"]).astype(np.float32)
    fw1 = np.zeros((2 * 97, 4 * D), np.float32)
    for ch in range(2):
        fw1[97 * ch:97 * ch + 96] = w1[96 * ch:96 * ch + 96]
        fw1[97 * ch + 96] = b1 * 0.5
    c["fw1"] = fw1
    fw2 = np.zeros((128, 6 * D), np.float32)
    for q in range(6):
        fw2[:, D * q:D * q + D] = inp["ff_w2"][128 * q:128 * q + 128, :]
    c["fw2"] = fw2
    c["fb2"] = np.broadcast_to(inp["ff_b2"][None], (128, D)).copy().astype(np.float32)
    hw = np.zeros((96, 2 * QL), np.float32)
    hw[:, 0:QL] = inp["head_w"][0:96]
    hw[:, QL:2 * QL] = inp["head_w"][96:192]
    c["hw"] = hw
    c["hb"] = np.broadcast_to(inp["head_b"][None], (128, QL)).copy().astype(np.float32)
    sel = np.zeros((128, 4), np.float32)
    for h in range(4):
        sel[32 * h + 24, h] = 1.0
    c["sel"] = sel
    rp = np.zeros((32, 8 * 128), np.float32)
    for k in range(8):
        for h in range(4):
            rp[4 * k + h, 128 * k + 32 * h:128 * k + 32 * h + 32] = 1.0
    c["repl"] = rp
    return c


def _core_inputs(inp, const, core):
    """Per-core in_map (includes the shared consts)."""
    m = dict(const)
    qg = np.asarray(inp["query_grid"], np.float32)
    tp = np.asarray(inp["t_pos_w"], np.float32)
    mt = np.asarray(inp["mem_tokens"], np.float32)
    qT = np.zeros((NSLOT * 193, NQ), np.float32)
    for j in range(NSLOT):
        b, t = _frame(8 * j + core)
        qT[193 * j:193 * j + 192] = (qg + tp[t][None, :]).T
        qT[193 * j + 192] = 1.0
        kvT = np.ones((193, KV[j]), np.float32)
        kvT[0:192] = mt[b, :TMAX[j]].reshape(-1, D).T
        m[f"kvT{j}"] = kvT
        msk = np.zeros((4, KV[j]), np.float32)
        msk[:, 128 * t:] = -CM
        m[f"msk{j}"] = msk
    m["qT"] = qT
    return m


# ---------------------------------------------------------------- program

_CACHE = {}

# consts whose DRAM row-count exceeds 128: load as (rows0:97|0:96, rest) pairs
_SPLIT193 = ("cwq0", "cwq1", "cwk0", "cwk1", "cwv")


def build_program(gelu_f=AF.Gelu, debug=False, stop_stage=4):
    key = (gelu_f, debug, stop_stage)
    if key in _CACHE:
        return _CACHE[key]
    nc = bacc.Bacc("TRN2", target_bir_lowering=False, debug=False)

    # ---- DRAM I/O ----
    dr = {}
    def din(name, shape):
        dr[name] = nc.dram_tensor(name, shape, F32, kind="ExternalInput")
    for gg in range(2):
        din(f"cwq{gg}", (193, 128)); din(f"cwk{gg}", (193, 128))
    din("cwv", (193, 256)); din("cwo", (128, 2 * D))
    for pre in ("r", "l"):
        for ch in range(2):
            for gg in range(2):
                din(f"{pre}wq{ch}{gg}", (97, 128)); din(f"{pre}wk{ch}{gg}", (97, 128))
            din(f"{pre}wv{ch}", (97, 256))
        din(f"{pre}wo", (128, 2 * D))
        din(f"{pre}ka", (128, NQ)); din(f"{pre}qa", (128, NQ))
    din("fw1", (2 * 97, 4 * D)); din("fw2", (128, 6 * D)); din("fb2", (128, D))
    din("hw", (96, 2 * QL)); din("hb", (128, QL))
    din("sel", (128, 4)); din("repl", (32, 8 * 128))
    din("qT", (NSLOT * 193, NQ))
    for j in range(NSLOT):
        din(f"kvT{j}", (193, KV[j])); din(f"msk{j}", (4, KV[j]))
    out = nc.dram_tensor("out", (NSLOT * 2 * 128, QL), F32, kind="ExternalOutput")
    dbg = {}
    if debug:
        for nm, shape in (("d_qhT", (128, 256)), ("d_khT", (128, 512)),
                          ("d_pT", (128, 1024)), ("d_av", (128, 256)),
                          ("d_avn", (128, 256)), ("d_r1", (128, D)),
                          ("d_xh", (128, D)), ("d_xT", (97, 256)),
                          ("d_r2", (128, D)), ("d_r3", (128, D)),
                          ("d_h1g", (128, 256)), ("d_z", (128, D))):
            dbg[nm] = nc.dram_tensor(nm, shape, F32, kind="ExternalOutput")

    from contextlib import ExitStack
    with tile.TileContext(nc) as tc, ExitStack() as es:
        cst = es.enter_context(tc.tile_pool(name="cst", bufs=1))
        dyn = es.enter_context(tc.tile_pool(name="dyn", bufs=2))
        dy3 = es.enter_context(tc.tile_pool(name="dy3", bufs=3))
        dy8 = es.enter_context(tc.tile_pool(name="dy8", bufs=8))
        st = es.enter_context(tc.tile_pool(name="st", bufs=1))
        ps_s = es.enter_context(tc.tile_pool(name="ps_s", bufs=2, space="PSUM"))
        ps_a = es.enter_context(tc.tile_pool(name="ps_a", bufs=2, space="PSUM"))
        ps_g = es.enter_context(tc.tile_pool(name="ps_g", bufs=2, space="PSUM"))

        def rr(ap):
            return ap.bitcast(mybir.dt.float32r)

        def pg(p_, f_):
            return ps_g.tile([p_, f_], F32, tag="pg", name="pg")

        # ---- load constants ----
        C = {}
        for nm, t_ in dr.items():
            if nm == "qT" or nm.startswith(("kvT", "msk")):
                continue
            shape = list(t_.shape)
            dt_ = t_.dtype
            if nm in _SPLIT193:
                ta = cst.tile([96, shape[1]], dt_, tag=nm + "a")
                tb = cst.tile([97, shape[1]], dt_, tag=nm + "b")
                nc.sync.dma_start(ta[:], t_.ap()[0:96, :])
                nc.sync.dma_start(tb[:], t_.ap()[96:193, :])
                C[nm] = (ta, tb)
            elif nm == "fw1":
                ta = cst.tile([97, shape[1]], dt_, tag="fw1a")
                tb = cst.tile([97, shape[1]], dt_, tag="fw1b")
                nc.sync.dma_start(ta[:], t_.ap()[0:97, :])
                nc.sync.dma_start(tb[:], t_.ap()[97:194, :])
                C[nm] = (ta, tb)
            else:
                tl = cst.tile(shape, dt_, tag=nm)
                nc.sync.dma_start(tl[:], t_.ap()[:])
                C[nm] = tl
        ident = cst.tile([128, 128], F32, tag="ident")
        make_identity(nc, ident[:])
        epsc = cst.tile([128, 1], F32, tag="epsc")
        nc.gpsimd.memset(epsc[:], EPS)

        def evac(dst_ap, src_ap):
            nc.vector.tensor_copy(dst_ap, src_ap)

        # ============ generic attention core ============
        def attn_core(iid, qsrc, ksrc, vsrc, kauxdma, qauxdma, kvlen, kp,
                      lstack, pres):
            """qsrc/ksrc: per half, list of (lhsT_chunk, rhs_chunk) pairs.
            vsrc(i): chunk pairs for kv-tile i. kp = rows/head (25 or 27).
            Returns av_sb (unnormalized ovT + l rows) per half."""
            nkt = kvlen // 128
            qhT, khT = [], []
            for g in range(2):
                pq = pg(128, 256)
                for i, (cm, rhs) in enumerate(qsrc[g]):
                    nc.tensor.matmul(pq[:], cm, rhs, start=(i == 0),
                                     stop=(i == len(qsrc[g]) - 1))
                q_sb = dyn.tile([128, 256], F32, tag=f"qhT{g}")
                evac(q_sb[:], pq[:])
                if qauxdma is not None:
                    for h in range(4):
                        nc.gpsimd.tensor_copy(q_sb[32 * h:32 * h + 3, :],
                                         qauxdma[32 * h:32 * h + 3, :])
                qhT.append(q_sb)
                k_sb = dyn.tile([128, kvlen], F32, tag=f"khT{g}", bufs=1)
                for nchunk in range(0, kvlen, 512):
                    nw = min(512, kvlen - nchunk)
                    pk = pg(128, 512)
                    for i, (cm, rhs) in enumerate(ksrc[g]):
                        nc.tensor.matmul(pk[:, 0:nw], cm,
                                         rhs[:, nchunk:nchunk + nw],
                                         start=(i == 0),
                                         stop=(i == len(ksrc[g]) - 1))
                    evac(k_sb[:, nchunk:nchunk + nw], pk[:, 0:nw])
                if kauxdma[1] == 1:       # stage-1 dynamic mask row (DRAM)
                    for h in range(4):
                        nc.gpsimd.dma_start(k_sb[32 * h:32 * h + 1, :],
                                            kauxdma[0][h:h + 1, :])
                else:                      # static axial-mask rows (SBUF)
                    for h in range(4):
                        nc.gpsimd.tensor_copy(k_sb[32 * h:32 * h + 3, :],
                                         kauxdma[0][32 * h:32 * h + 3, :])
                khT.append(k_sb)
            av = [ps_a.tile([128, 256], F32, tag="p_av", name="p_av") for _ in range(2)]
            npair = nkt // 2
            for ip in range(npair):
                vhs = []
                for d in range(2):
                    pv = pg(128, 256)
                    vch = vsrc(2 * ip + d)
                    for ci, (cm, rhs) in enumerate(vch):
                        nc.tensor.matmul(pv[:], cm, rhs, start=(ci == 0),
                                         stop=(ci == len(vch) - 1))
                    vh = dy3.tile([128, 256], F32, tag="vh")
                    evac(vh[:], pv[:])
                    vhs.append(vh)
                for g in range(2):
                    # head h in its own PSUM bank (cols 512h) -- concurrent
                    # row-group matmuls to one bank collide fatally on HW
                    pssT = ps_s.tile([128, 2048], F32, tag="p_sT", bufs=1)
                    for d in range(2):
                        for h in range(4):
                            nc.tensor.matmul(
                                pssT[:, 512 * h + 256 * d:512 * h + 256 * d + 256],
                                khT[g][32 * h:32 * h + kp,
                                       128 * (2 * ip + d):128 * (2 * ip + d) + 128],
                                qhT[g][32 * h:32 * h + kp, :],
                                start=True, stop=True, tile_position=(32 * h, 0))
                    pT = dy3.tile([128, 2048], F32, tag="pT", bufs=2)
                    nc.scalar.activation(pT[:], pssT[:], AF.Exp)
                    if debug and iid == 0 and ip == 0 and g == 0 and pres == "s1":
                        nc.gpsimd.dma_start(dbg["d_pT"].ap()[:], pT[:, 0:1024])
                    for d in range(2):
                        for h in range(4):
                            nc.tensor.matmul(
                                av[g][32 * h:32 * h + 32, :],
                                vhs[d][:, 32 * (4 * g + h):32 * (4 * g + h) + 32],
                                pT[:, 512 * h + 256 * d:512 * h + 256 * d + 256],
                                start=(ip == 0 and d == 0),
                                stop=(ip == npair - 1 and d == 1),
                                tile_position=(0, 32 * h), skip_group_check=True)
            av_sb = []
            for g in range(2):
                a_sb = dy8.tile([128, 256], F32, tag="av_sb")
                evac(a_sb[:], av[g][:])
                pl = pg(4, 256)
                nc.tensor.matmul(pl[:], C["sel"][:, 0:4], a_sb[:],
                                 start=True, stop=True)
                ltmp = dy3.tile([4, 256], F32, tag="ltmp")
                evac(ltmp[:], pl[:])
                nc.gpsimd.dma_start(lstack[8 * iid + 4 * g:8 * iid + 4 * g + 4, :],
                                    ltmp[:])
                av_sb.append(a_sb)
            if debug and iid == 0 and pres == "s1":
                nc.gpsimd.dma_start(dbg["d_qhT"].ap()[:], qhT[0][:])
                nc.gpsimd.dma_start(dbg["d_khT"].ap()[:], khT[0][:, 0:512])
                nc.sync.dma_start(dbg["d_av"].ap()[:], av_sb[0][:])
            return av_sb

        def attn_finish(iid, av_sb, rstack, wo_t, res_in, res_tag):
            """r-broadcast, normalize, wo projection (+residual)."""
            avn = []
            for g in range(2):
                prb = pg(128, 256)
                k = 2 * iid + g
                nc.tensor.matmul(prb[:], C["repl"][:, 128 * k:128 * k + 128],
                                 rstack[:], start=True, stop=True)
                an = dy3.tile([128, 256], F32, tag="avn")
                nc.vector.tensor_tensor(an[:], av_sb[g][:], prb[:], OP.mult)
                avn.append(an)
            outs = []
            for tt in range(2):
                py = pg(128, D)
                for g in range(2):
                    nc.tensor.matmul(
                        py[:], avn[g][:, 128 * tt:128 * tt + 128],
                        wo_t[:, D * g:D * g + D],
                        start=(g == 0), stop=(g == 1))
                r_new = st.tile([128, D], F32, tag=f"{res_tag}_{iid}_{tt}")
                if res_in is None:
                    evac(r_new[:], py[:])
                else:
                    nc.vector.tensor_tensor(r_new[:], res_in[tt][:], py[:], OP.add)
                outs.append(r_new)
            return outs

        # ============ LN helpers ============
        def ln_stats(x_tiles, name):
            # tensor_tensor_reduce is fatal at runtime on this stack; use
            # bn_stats/bn_aggr (one DVE pass -> mean, var per partition)
            n = len(x_tiles)
            var = dyn.tile([128, n], F32, tag=f"var_{name}")
            rs = dyn.tile([128, n], F32, tag=f"rs_{name}")
            aggs = []
            for k, xt in enumerate(x_tiles):
                bst = dy3.tile([128, 6], F32, tag="bst")
                nc.vector.bn_stats(bst[:], xt[:])
                agg = dy8.tile([128, 2], F32, tag=f"agg_{name}", name="agg")
                nc.vector.bn_aggr(agg[:], bst[:])
                nc.vector.tensor_copy(var[:, k:k + 1], agg[:, 1:2])
                aggs.append(agg)
            lnv = dyn.tile([128, n], F32, tag=f"lnv_{name}")
            nc.scalar.activation(lnv[:], var[:], AF.Ln, bias=epsc[:])
            nc.scalar.activation(rs[:], lnv[:], AF.Exp, scale=-0.5)
            return aggs, rs

        def ln_apply(x, aggs, rs, k, name):
            xh = dy3.tile([128, D], F32, tag=f"xh_{name}")
            nc.vector.tensor_scalar(xh[:], x[:], aggs[k][:, 0:1], rs[:, k:k + 1],
                                    OP.subtract, OP.mult)
            return xh

        def transpose_pair(xh_tiles, name, ones_row=True):
            xT = []
            for ch in range(2):
                t_ = dyn.tile([97, 256], F32, tag=f"xT{ch}", name=f"xT{ch}")
                for tt in range(2):
                    pt = pg(96, 128)
                    nc.tensor.transpose(pt[:], xh_tiles[tt][:, 96 * ch:96 * ch + 96],
                                        ident[:])
                    evac(t_[0:96, 128 * tt:128 * tt + 128], pt[:])
                if ones_row:
                    nc.gpsimd.memset(t_[96:97, :], 1.0)
                xT.append(t_)
            return xT

        # ================= stage 1: cross attention =================
        lstack1 = st.tile([32, 256], F32, tag="lstack1")
        av1 = []
        for j in range(NSLOT):
            kva = dyn.tile([96, KV[j]], F32, tag="kvTa", bufs=2)
            kvb = dyn.tile([97, KV[j]], F32, tag="kvTb", bufs=2)
            nc.sync.dma_start(kva[:], dr[f"kvT{j}"].ap()[0:96, :])
            nc.sync.dma_start(kvb[:], dr[f"kvT{j}"].ap()[96:193, :])
            qta = dyn.tile([96, 256], F32, tag="qTa")
            qtb = dyn.tile([97, 256], F32, tag="qTb")
            nc.sync.dma_start(qta[:], dr["qT"].ap()[193 * j:193 * j + 96, :])
            nc.sync.dma_start(qtb[:], dr["qT"].ap()[193 * j + 96:193 * j + 193, :])
            qsrc = [[(C[f"cwq{g}"][0][:], qta[:]), (C[f"cwq{g}"][1][:], qtb[:])]
                    for g in range(2)]
            ksrc = [[(C[f"cwk{g}"][0][:], kva[:]), (C[f"cwk{g}"][1][:], kvb[:])]
                    for g in range(2)]
            def vsrc(i, kva=kva, kvb=kvb):
                return [(kva[:, 128 * i:128 * i + 128], C["cwv"][0][:]),
                        (kvb[:, 128 * i:128 * i + 128], C["cwv"][1][:])]
            av_sb = attn_core(j, qsrc, ksrc, vsrc,
                              (dr[f"msk{j}"].ap()[:], 1), None,
                              KV[j], 25, lstack1, "s1")
            av1.append(av_sb)
        rstack1 = st.tile([32, 256], F32, tag="rstack1")
        nc.vector.reciprocal(rstack1[:], lstack1[:])
        r1 = []
        for j in range(NSLOT):
            r1.append(attn_finish(j, av1[j], rstack1, C["cwo"], None, "r1"))
        if debug:
            nc.sync.dma_start(dbg["d_avn"].ap()[:], av1[0][0][:])
            nc.sync.dma_start(dbg["d_r1"].ap()[:], r1[0][0][:])

        # ================= stages 2 (row) and 3 (col) =================
        def axial_stage(pre, res, res_tag, sname):
            x_all = [res[f][tt] for f in range(NSLOT) for tt in range(2)]
            aggs, rs = ln_stats(x_all, sname)
            lst = st.tile([32, 256], F32, tag=f"lstack_{sname}")
            avs = []
            for f in range(NSLOT):
                xh = [ln_apply(res[f][tt], aggs, rs, 2 * f + tt, sname)
                      for tt in range(2)]
                xT = transpose_pair(xh, sname)
                if debug and f == 0 and pre == "r":
                    nc.sync.dma_start(dbg["d_xh"].ap()[:], xh[0][:])
                    nc.gpsimd.dma_start(dbg["d_xT"].ap()[:], xT[0][:])
                qsrc = [[(C[f"{pre}wq{ch}{g}"][:], xT[ch][:]) for ch in range(2)]
                        for g in range(2)]
                ksrc = [[(C[f"{pre}wk{ch}{g}"][:], xT[ch][:]) for ch in range(2)]
                        for g in range(2)]
                def vsrc(i, xT=xT):
                    return [(xT[ch][0:97, 128 * i:128 * i + 128],
                             C[f"{pre}wv{ch}"][:]) for ch in range(2)]
                av_sb = attn_core(f, qsrc, ksrc, vsrc,
                                  (C[f"{pre}ka"][:], 3),
                                  C[f"{pre}qa"][:],
                                  NQ, 27, lst, pre)
                avs.append(av_sb)
            rst = st.tile([32, 256], F32, tag=f"rstack_{sname}")
            nc.vector.reciprocal(rst[:], lst[:])
            return [attn_finish(f, avs[f], rst, C[f"{pre}wo"], res[f], res_tag)
                    for f in range(NSLOT)]

        def dump_partial(res):
            for f in range(NSLOT):
                for tt in range(2):
                    row = 128 * (2 * f + tt)
                    nc.gpsimd.dma_start(out.ap()[row:row + 128, 0:D], res[f][tt][:])

        if stop_stage == 1:
            dump_partial(r1)
            r2 = None
        else:
            r2 = axial_stage("r", r1, "r2", "s2")
        if debug and r2 is not None:
            nc.sync.dma_start(dbg["d_r2"].ap()[:], r2[0][0][:])
        if stop_stage == 2 and r2 is not None:
            dump_partial(r2)
        r3 = axial_stage("l", r2, "r3", "s3") if stop_stage >= 3 else None
        if debug and r3 is not None:
            nc.sync.dma_start(dbg["d_r3"].ap()[:], r3[0][0][:])

        # ================= stage 4: FFN + head =================
        if stop_stage == 3 and r3 is not None:
            dump_partial(r3)
        x_all = [r3[f][tt] for f in range(NSLOT) for tt in range(2)] \
            if stop_stage >= 4 else []
        aggs4, rs4 = ln_stats(x_all, "s4") if stop_stage >= 4 else (None, None)
        for f in range(NSLOT if stop_stage >= 4 else 0):
            xh = [ln_apply(r3[f][tt], aggs4, rs4, 2 * f + tt, "s4")
                  for tt in range(2)]
            xT = transpose_pair(xh, "s4")
            h1g = []
            for q in range(6):
                ph = pg(128, 256)
                for ch in range(2):
                    nc.tensor.matmul(ph[:],
                                     C["fw1"][ch][:, 128 * q:128 * q + 128],
                                     xT[ch][:], start=(ch == 0), stop=(ch == 1))
                hg = dyn.tile([128, 256], F32, tag=f"h1g{q}")
                nc.scalar.activation(hg[:], ph[:], gelu_f)
                h1g.append(hg)
            if debug and f == 0:
                nc.gpsimd.dma_start(dbg["d_h1g"].ap()[:], h1g[0][:])
            z = []
            for tt in range(2):
                pz = pg(128, D)
                for q in range(6):
                    nc.tensor.matmul(pz[:], h1g[q][:, 128 * tt:128 * tt + 128],
                                     C["fw2"][:, D * q:D * q + D],
                                     start=(q == 0), stop=(q == 5))
                zt = dy3.tile([128, D], F32, tag="z_t")
                nc.vector.tensor_tensor(zt[:], pz[:], C["fb2"][:], OP.add)
                z_sb = st.tile([128, D], F32, tag=f"z_{f}_{tt}")
                nc.vector.tensor_tensor(z_sb[:], zt[:], r3[f][tt][:], OP.add)
                z.append(z_sb)
            if debug and f == 0:
                nc.sync.dma_start(dbg["d_z"].ap()[:], z[0][:])
            zT = transpose_pair(z, "hz", ones_row=False)
            for tt in range(2):
                po = pg(128, QL)
                for ch in range(2):
                    nc.tensor.matmul(po[:], zT[ch][0:96, 128 * tt:128 * tt + 128],
                                     C["hw"][:, QL * ch:QL * ch + QL],
                                     start=(ch == 0), stop=(ch == 1))
                ot = dy3.tile([128, QL], F32, tag="o_t")
                nc.vector.tensor_tensor(ot[:], po[:], C["hb"][:], OP.add)
                row = 128 * (2 * f + tt)
                nc.gpsimd.dma_start(out.ap()[row:row + 128, :], ot[:])

    nc.compile()
    _CACHE[key] = nc
    return nc


# ---------------------------------------------------------------- entry

def kernel(**inputs):
    inputs = {k: np.asarray(v, np.float32) for k, v in inputs.items()}
    nc = build_program()
    const = _host_constants(inputs)
    in_maps = [_core_inputs(inputs, const, c) for c in range(NCORE)]
    res = bass_utils.run_bass_kernel_spmd(nc, in_maps, core_ids=list(range(NCORE)))
    out = np.zeros((B, T - 1, H, W, QL), np.float32)
    for f in range(30):
        b, t = _frame(f)
        core, j = f % 8, f // 8
        o = res.results[core]["out"].reshape(NSLOT, 2 * 128, QL)
        out[b, t - 1] = o[j].reshape(H, W, QL)
    return out



# revision 33
# speedup vs baseline: 2.0911x; 2.0911x over previous
"""Trainium2 Bass kernel for nn_DecoderVectorized (axial decoder with causal
cross-attention). Self-contained: hardcodes all shapes/sharding.

Sharding: 32 SPMD slots = 8 cores x 4 slots over the B*(T-1)=30 frames
(sorted by t so slot j has kv prefix 128*{4,8,12,15}; 2 dummy slots).
"""
import math
import sys

import numpy as np

try:
    import concourse.bass as bass
except ImportError:  # pragma: no cover
    sys.path.insert(0, "/opt/trn_rl_repo")
    import concourse.bass as bass

import concourse.bacc as bacc
import concourse.mybir as mybir
import concourse.tile as tile
from concourse import bass_utils
from concourse.masks import make_identity

F32 = mybir.dt.float32
F32R = mybir.dt.float32r
BF16 = mybir.dt.bfloat16
import ml_dtypes
NPBF = ml_dtypes.bfloat16
AF = mybir.ActivationFunctionType
OP = mybir.AluOpType

H, W, D, HEADS, QL = 16, 16, 192, 8, 256
B, T, M = 2, 16, 128
NQ = H * W          # 256 tokens per frame
DH = D // HEADS     # 24
NCORE, NSLOT = 8, 4
TMAX = [4, 8, 12, 16]
KV = [t * 128 for t in TMAX]        # 512 1024 1536 2048
CM = 32.0                           # mask bias (power of 2: bf16-exact)
SCL = 1.0 / math.sqrt(DH)
EPS = 1e-5


def _frame(f):
    """frame index f in [0,32) -> (b, t); 30/31 are dummies."""
    if f >= 30:
        return (f - 30, 15)
    return (f % 2, f // 2 + 1)


# ---------------------------------------------------------------- host prep

def _qk_colmat_s1(w, bvec, g, is_q):
    """[193,128] colmat for stage-1 qhT/khT half g. Head h data at cols
    32h+1..32h+24 (aux row 0 first); col 32h+0: e_192 (ones) if is_q."""
    m = np.zeros((193, 128), np.float32)
    for h in range(4):
        Hh = 4 * g + h
        m[0:192, 32 * h + 1:32 * h + 25] = w[:, DH * Hh:DH * Hh + DH]
        m[192, 32 * h + 1:32 * h + 25] = bvec[DH * Hh:DH * Hh + DH]
        if is_q:
            m[192, 32 * h] = 1.0
    return m


def _qk_colmat_23(w, bvec, ch, g):
    """[97,128] colmat chunk for stage-2/3 qhT/khT. Data at cols 32h+3..+26;
    aux cols 0..2 zero (DMA'd)."""
    m = np.zeros((97, 128), np.float32)
    for h in range(4):
        Hh = 4 * g + h
        m[0:96, 32 * h + 3:32 * h + 27] = w[96 * ch:96 * ch + 96, DH * Hh:DH * Hh + DH]
        m[96, 32 * h + 3:32 * h + 27] = bvec[DH * Hh:DH * Hh + DH] * 0.5
    return m


def _wv_colmat_s1(w, bvec):
    """[193,256]: head Hh data at cols 32Hh..+23, ones col at 32Hh+24."""
    m = np.zeros((193, 256), np.float32)
    for Hh in range(8):
        m[0:192, 32 * Hh:32 * Hh + 24] = w[:, DH * Hh:DH * Hh + DH]
        m[192, 32 * Hh:32 * Hh + 24] = bvec[DH * Hh:DH * Hh + DH]
        m[192, 32 * Hh + 24] = 1.0
    return m


def _wv_colmat_23(w, bvec, ch):
    m = np.zeros((97, 256), np.float32)
    for Hh in range(8):
        m[0:96, 32 * Hh:32 * Hh + 24] = w[96 * ch:96 * ch + 96, DH * Hh:DH * Hh + DH]
        m[96, 32 * Hh:32 * Hh + 24] = bvec[DH * Hh:DH * Hh + DH] * 0.5
        m[96, 32 * Hh + 24] = 0.5
    return m


def _wo_aug(w, bvec):
    """[128, 384]: head H=4g+h at partition rows 32h..32h+31, col block 192g:
    rows 0..23 = wo rows, row 24 = bo/8 (walrus needs lhsT/rhs same base)."""
    m = np.zeros((128, 2 * D), np.float32)
    for g in range(2):
        for h in range(4):
            Hh = 4 * g + h
            m[32 * h:32 * h + 24, D * g:D * g + D] = w[DH * Hh:DH * Hh + DH, :]
            m[32 * h + 24, D * g:D * g + D] = bvec / 8.0
    return m


def _aux_rows(idx, is_q):
    """[128,256] aux contraction rows for the rank-3 axial mask, pre-spread to
    partition rows 32h+0..2. k-side: [ri^2, ri, 1]; q-side: [-c, 2c rj, -c rj^2]."""
    r = idx.astype(np.float32)
    if is_q:
        rows = np.stack([np.full(NQ, -CM, np.float32), 2.0 * CM * r, -CM * r * r])
    else:
        rows = np.stack([r * r, r, np.ones(NQ, np.float32)])
    m = np.zeros((128, NQ), np.float32)
    for h in range(4):
        m[32 * h:32 * h + 3] = rows
    return m


def _host_constants(inp):
    """Shared (core-independent) device constant arrays."""
    c = {}
    g, b_ = inp["rn_g"], inp["rn_b"]

    def eff(wq, bq, scale):
        return (g[:, None] * wq * scale).astype(np.float32), \
               ((b_ @ wq + bq) * scale).astype(np.float32)

    for gg in range(2):
        c[f"cwq{gg}"] = _qk_colmat_s1(inp["c_wq"] * SCL, inp["c_bq"] * SCL, gg, True)
        c[f"cwk{gg}"] = _qk_colmat_s1(inp["c_wk"], inp["c_bk"], gg, False)
    c["cwv"] = _wv_colmat_s1(inp["c_wv"], inp["c_bv"])
    c["cwo"] = _wo_aug(inp["c_wo"], inp["c_bo"])
    tok = np.arange(NQ)
    for pre, wp, idx in (("r", "r", tok // 16), ("l", "col", tok % 16)):
        wq, bq = eff(inp[wp + "_wq"], inp[wp + "_bq"], SCL)
        wk, bk = eff(inp[wp + "_wk"], inp[wp + "_bk"], 1.0)
        wv, bv = eff(inp[wp + "_wv"], inp[wp + "_bv"], 1.0)
        for ch in range(2):
            for gg in range(2):
                c[f"{pre}wq{ch}{gg}"] = _qk_colmat_23(wq, bq, ch, gg)
                c[f"{pre}wk{ch}{gg}"] = _qk_colmat_23(wk, bk, ch, gg)
            c[f"{pre}wv{ch}"] = _wv_colmat_23(wv, bv, ch)
        c[f"{pre}wo"] = _wo_aug(inp[wp + "_wo"], inp[wp + "_bo"])
        c[f"{pre}ka"] = _aux_rows(idx, False)
        c[f"{pre}qa"] = _aux_rows(idx, True)
    w1 = (inp["ff_ln_g"][:, None] * inp["ff_w1"]).astype(np.float32)
    b1 = (inp["ff_ln_b"] @ inp["ff_w1"] + inp["ff_b1"]).astype(np.float32)
    fw1 = np.zeros((2 * 97, 4 * D), np.float32)
    for ch in range(2):
        fw1[97 * ch:97 * ch + 96] = w1[96 * ch:96 * ch + 96]
        fw1[97 * ch + 96] = b1 * 0.5
    c["fw1"] = fw1
    fw2 = np.zeros((128, 6 * D), np.float32)
    for q in range(6):
        fw2[:, D * q:D * q + D] = inp["ff_w2"][128 * q:128 * q + 128, :]
    c["fw2"] = fw2
    c["fb2"] = np.broadcast_to(inp["ff_b2"][None], (128, D)).copy().astype(np.float32)
    hw = np.zeros((96, 2 * QL), np.float32)
    hw[:, 0:QL] = inp["head_w"][0:96]
    hw[:, QL:2 * QL] = inp["head_w"][96:192]
    c["hw"] = hw
    c["hb"] = np.broadcast_to(inp["head_b"][None], (128, QL)).copy().astype(np.float32)
    rp = np.zeros((32, 8 * 128), np.float32)
    for k in range(8):
        for h in range(4):
            rp[4 * k + h, 128 * k + 32 * h:128 * k + 32 * h + 32] = 1.0
    c["repl"] = rp
    # all matmul operands stream as bf16 (1 PE cycle/row vs 4 for fp32)
    for nm in c:
        if nm not in ("fb2", "hb"):
            c[nm] = c[nm].astype(NPBF)
    return c


def _core_inputs(inp, const, core):
    """Per-core in_map (includes the shared consts)."""
    m = dict(const)
    qg = np.asarray(inp["query_grid"], np.float32)
    tp = np.asarray(inp["t_pos_w"], np.float32)
    mt = np.asarray(inp["mem_tokens"], np.float32)
    qT = np.zeros((NSLOT * 193, NQ), np.float32)
    for j in range(NSLOT):
        b, t = _frame(8 * j + core)
        qT[193 * j:193 * j + 192] = (qg + tp[t][None, :]).T
        qT[193 * j + 192] = 1.0
        kvT = np.ones((193, KV[j]), np.float32)
        kvT[0:192] = mt[b, :TMAX[j]].reshape(-1, D).T
        m[f"kvT{j}"] = kvT.astype(NPBF)
        msk = np.zeros((4, KV[j]), np.float32)
        msk[:, 128 * t:] = -CM
        m[f"msk{j}"] = msk.astype(NPBF)
    m["qT"] = qT.astype(NPBF)
    return m


# ---------------------------------------------------------------- program

_CACHE = {}

# consts whose DRAM row-count exceeds 128: load as (rows0:97|0:96, rest) pairs
_SPLIT193 = ("cwq0", "cwq1", "cwk0", "cwk1", "cwv")


def build_program(gelu_f=AF.Gelu, debug=False, stop_stage=4):
    key = (gelu_f, debug, stop_stage)
    if key in _CACHE:
        return _CACHE[key]
    nc = bacc.Bacc("TRN2", target_bir_lowering=False, debug=False)

    # ---- DRAM I/O ----
    dr = {}
    def din(name, shape, dt_=BF16):
        dr[name] = nc.dram_tensor(name, shape, dt_, kind="ExternalInput")
    for gg in range(2):
        din(f"cwq{gg}", (193, 128)); din(f"cwk{gg}", (193, 128))
    din("cwv", (193, 256)); din("cwo", (128, 2 * D))
    for pre in ("r", "l"):
        for ch in range(2):
            for gg in range(2):
                din(f"{pre}wq{ch}{gg}", (97, 128)); din(f"{pre}wk{ch}{gg}", (97, 128))
            din(f"{pre}wv{ch}", (97, 256))
        din(f"{pre}wo", (128, 2 * D))
        din(f"{pre}ka", (128, NQ)); din(f"{pre}qa", (128, NQ))
    din("fw1", (2 * 97, 4 * D)); din("fw2", (128, 6 * D))
    din("fb2", (128, D), F32)
    din("hw", (96, 2 * QL)); din("hb", (128, QL), F32)
    din("repl", (32, 8 * 128))
    din("qT", (NSLOT * 193, NQ))
    for j in range(NSLOT):
        din(f"kvT{j}", (193, KV[j])); din(f"msk{j}", (4, KV[j]))
    out = nc.dram_tensor("out", (NSLOT * 2 * 128, QL), F32, kind="ExternalOutput")
    dbg = {}
    if debug:
        for nm, shape, dt_ in (("d_qhT", (128, 256), BF16), ("d_khT", (128, 512), BF16),
                               ("d_pT", (128, 1024), BF16), ("d_av", (128, 256), F32),
                               ("d_avn", (128, 256), F32), ("d_r1", (128, D), F32),
                               ("d_xh", (128, D), BF16), ("d_xT", (97, 256), BF16),
                               ("d_r2", (128, D), F32), ("d_r3", (128, D), F32),
                               ("d_h1g", (128, 256), BF16), ("d_z", (128, D), BF16)):
            dbg[nm] = nc.dram_tensor(nm, shape, dt_, kind="ExternalOutput")

    from contextlib import ExitStack
    with tile.TileContext(nc) as tc, ExitStack() as es, \
            nc.allow_low_precision(reason="bf16 operands; rel-err gate 2e-2"):
        cst = es.enter_context(tc.tile_pool(name="cst", bufs=1))
        dyn = es.enter_context(tc.tile_pool(name="dyn", bufs=2))
        dy3 = es.enter_context(tc.tile_pool(name="dy3", bufs=3))
        dy8 = es.enter_context(tc.tile_pool(name="dy8", bufs=8))
        st = es.enter_context(tc.tile_pool(name="st", bufs=1))
        ps_s = es.enter_context(tc.tile_pool(name="ps_s", bufs=2, space="PSUM"))
        ps_a = es.enter_context(tc.tile_pool(name="ps_a", bufs=2, space="PSUM"))
        ps_g = es.enter_context(tc.tile_pool(name="ps_g", bufs=2, space="PSUM"))

        def rr(ap):
            return ap.bitcast(mybir.dt.float32r)

        def pg(p_, f_, dt_=F32):
            return ps_g.tile([p_, f_], dt_, tag="pg", name="pg")

        # ---- load constants ----
        C = {}
        for nm, t_ in dr.items():
            if nm == "qT" or nm.startswith(("kvT", "msk")):
                continue
            shape = list(t_.shape)
            dt_ = t_.dtype
            if nm in _SPLIT193:
                ta = cst.tile([96, shape[1]], dt_, tag=nm + "a")
                tb = cst.tile([97, shape[1]], dt_, tag=nm + "b")
                nc.sync.dma_start(ta[:], t_.ap()[0:96, :])
                nc.sync.dma_start(tb[:], t_.ap()[96:193, :])
                C[nm] = (ta, tb)
            elif nm == "fw1":
                ta = cst.tile([97, shape[1]], dt_, tag="fw1a")
                tb = cst.tile([97, shape[1]], dt_, tag="fw1b")
                nc.sync.dma_start(ta[:], t_.ap()[0:97, :])
                nc.sync.dma_start(tb[:], t_.ap()[97:194, :])
                C[nm] = (ta, tb)
            else:
                tl = cst.tile(shape, dt_, tag=nm)
                nc.sync.dma_start(tl[:], t_.ap()[:])
                C[nm] = tl
        ident = cst.tile([128, 128], BF16, tag="ident")
        make_identity(nc, ident[:])
        epsc = cst.tile([128, 1], F32, tag="epsc")
        nc.gpsimd.memset(epsc[:], EPS)

        def evac(dst_ap, src_ap):
            nc.vector.tensor_copy(dst_ap, src_ap)

        # ============ generic attention core ============
        def attn_core(iid, qsrc, ksrc, vsrc, kauxdma, qauxdma, kvlen, kp,
                      lstack, pres):
            """qsrc/ksrc: per half, list of (lhsT_chunk, rhs_chunk) pairs.
            vsrc(i): chunk pairs for kv-tile i. kp = rows/head (25 or 27).
            Returns av_sb (unnormalized ovT + l rows) per half."""
            nkt = kvlen // 128
            qhT, khT = [], []
            for g in range(2):
                pq = pg(128, 256)
                for i, (cm, rhs) in enumerate(qsrc[g]):
                    nc.tensor.matmul(pq[:], cm, rhs, start=(i == 0),
                                     stop=(i == len(qsrc[g]) - 1))
                q_sb = dyn.tile([128, 256], BF16, tag=f"qhT{g}")
                evac(q_sb[:], pq[:])
                if qauxdma is not None:
                    for h in range(4):
                        nc.gpsimd.tensor_copy(q_sb[32 * h:32 * h + 3, :],
                                         qauxdma[32 * h:32 * h + 3, :])
                qhT.append(q_sb)
                k_sb = dyn.tile([128, kvlen], BF16, tag=f"khT{g}", bufs=1)
                for nchunk in range(0, kvlen, 512):
                    nw = min(512, kvlen - nchunk)
                    pk = pg(128, 512)
                    for i, (cm, rhs) in enumerate(ksrc[g]):
                        nc.tensor.matmul(pk[:, 0:nw], cm,
                                         rhs[:, nchunk:nchunk + nw],
                                         start=(i == 0),
                                         stop=(i == len(ksrc[g]) - 1))
                    evac(k_sb[:, nchunk:nchunk + nw], pk[:, 0:nw])
                if kauxdma[1] == 1:       # stage-1 dynamic mask row (DRAM)
                    for h in range(4):
                        nc.gpsimd.dma_start(k_sb[32 * h:32 * h + 1, :],
                                            kauxdma[0][h:h + 1, :])
                else:                      # static axial-mask rows (SBUF)
                    for h in range(4):
                        nc.gpsimd.tensor_copy(k_sb[32 * h:32 * h + 3, :],
                                         kauxdma[0][32 * h:32 * h + 3, :])
                khT.append(k_sb)
            av = [ps_a.tile([128, 256], F32, tag="p_av", name="p_av") for _ in range(2)]
            npair = nkt // 2
            for ip in range(npair):
                vhs = []
                for d in range(2):
                    pv = pg(128, 256)
                    vch = vsrc(2 * ip + d)
                    for ci, (cm, rhs) in enumerate(vch):
                        nc.tensor.matmul(pv[:], cm, rhs, start=(ci == 0),
                                         stop=(ci == len(vch) - 1))
                    vh = dy3.tile([128, 256], BF16, tag="vh")
                    evac(vh[:], pv[:])
                    vhs.append(vh)
                for g in range(2):
                    # head h in its own PSUM bank (cols 512h) -- concurrent
                    # row-group matmuls to one bank collide fatally on HW
                    pssT = ps_s.tile([128, 2048], F32, tag="p_sT", bufs=1)
                    for d in range(2):
                        for h in range(4):
                            nc.tensor.matmul(
                                pssT[:, 512 * h + 256 * d:512 * h + 256 * d + 256],
                                khT[g][32 * h:32 * h + kp,
                                       128 * (2 * ip + d):128 * (2 * ip + d) + 128],
                                qhT[g][32 * h:32 * h + kp, :],
                                start=True, stop=True, tile_position=(32 * h, 0))
                    pT = dy3.tile([128, 2048], BF16, tag="pT", bufs=2)
                    nc.scalar.activation(pT[:], pssT[:], AF.Exp)
                    if debug and iid == 0 and ip == 0 and g == 0 and pres == "s1":
                        nc.gpsimd.dma_start(dbg["d_pT"].ap()[:], pT[:, 0:1024])
                    for d in range(2):
                        for h in range(4):
                            nc.tensor.matmul(
                                av[g][32 * h:32 * h + 32, :],
                                vhs[d][:, 32 * (4 * g + h):32 * (4 * g + h) + 32],
                                pT[:, 512 * h + 256 * d:512 * h + 256 * d + 256],
                                start=(ip == 0 and d == 0),
                                stop=(ip == npair - 1 and d == 1),
                                tile_position=(0, 32 * h), skip_group_check=True)
            av_sb = []
            for g in range(2):
                a_sb = dy8.tile([128, 256], F32, tag="av_sb")
                evac(a_sb[:], av[g][:])
                # l rows live at partitions 32h+24: strided DMA extracts all 4
                nc.gpsimd.dma_start(lstack[8 * iid + 4 * g:8 * iid + 4 * g + 4, :],
                                    a_sb[24:121:32, :])
                av_sb.append(a_sb)
            if debug and iid == 0 and pres == "s1":
                nc.gpsimd.dma_start(dbg["d_qhT"].ap()[:], qhT[0][:])
                nc.gpsimd.dma_start(dbg["d_khT"].ap()[:], khT[0][:, 0:512])
                nc.sync.dma_start(dbg["d_av"].ap()[:], av_sb[0][:])
            return av_sb

        def attn_finish(iid, av_sb, rstack, wo_t, res_in, res_tag):
            """r-broadcast, normalize, wo projection (+residual)."""
            avn = []
            for g in range(2):
                prb = pg(128, 256)
                k = 2 * iid + g
                nc.tensor.matmul(prb[:], C["repl"][:, 128 * k:128 * k + 128],
                                 rstack[:], start=True, stop=True)
                an = dy3.tile([128, 256], BF16, tag="avn")
                nc.vector.tensor_tensor(an[:], av_sb[g][:], prb[:], OP.mult)
                avn.append(an)
            outs = []
            for tt in range(2):
                py = pg(128, D)
                for g in range(2):
                    nc.tensor.matmul(
                        py[:], avn[g][:, 128 * tt:128 * tt + 128],
                        wo_t[:, D * g:D * g + D],
                        start=(g == 0), stop=(g == 1))
                r_new = st.tile([128, D], F32, tag=f"{res_tag}_{iid}_{tt}")
                if res_in is None:
                    evac(r_new[:], py[:])
                else:
                    nc.vector.tensor_tensor(r_new[:], res_in[tt][:], py[:], OP.add)
                outs.append(r_new)
            return outs

        # ============ LN helpers ============
        def ln_stats(x_tiles, name):
            # tensor_tensor_reduce is fatal at runtime on this stack; use
            # bn_stats/bn_aggr (one DVE pass -> mean, var per partition)
            n = len(x_tiles)
            var = dyn.tile([128, n], F32, tag=f"var_{name}")
            rs = dyn.tile([128, n], F32, tag=f"rs_{name}")
            aggs = []
            for k, xt in enumerate(x_tiles):
                bst = dy3.tile([128, 6], F32, tag="bst")
                nc.vector.bn_stats(bst[:], xt[:])
                agg = dy8.tile([128, 2], F32, tag=f"agg_{name}", name="agg")
                nc.vector.bn_aggr(agg[:], bst[:])
                nc.vector.tensor_copy(var[:, k:k + 1], agg[:, 1:2])
                aggs.append(agg)
            lnv = dyn.tile([128, n], F32, tag=f"lnv_{name}")
            nc.scalar.activation(lnv[:], var[:], AF.Ln, bias=epsc[:])
            nc.scalar.activation(rs[:], lnv[:], AF.Exp, scale=-0.5)
            return aggs, rs

        def ln_apply(x, aggs, rs, k, name):
            xh = dy3.tile([128, D], BF16, tag=f"xh_{name}")
            nc.vector.tensor_scalar(xh[:], x[:], aggs[k][:, 0:1], rs[:, k:k + 1],
                                    OP.subtract, OP.mult)
            return xh

        def transpose_pair(xh_tiles, name, ones_row=True):
            xT = []
            for ch in range(2):
                t_ = dyn.tile([97, 256], BF16, tag=f"xT{ch}", name=f"xT{ch}")
                for tt in range(2):
                    pt = pg(96, 128, BF16)
                    nc.tensor.transpose(pt[:], xh_tiles[tt][:, 96 * ch:96 * ch + 96],
                                        ident[:])
                    evac(t_[0:96, 128 * tt:128 * tt + 128], pt[:])
                if ones_row:
                    nc.gpsimd.memset(t_[96:97, :], 1.0)
                xT.append(t_)
            return xT

        # ================= stage 1: cross attention =================
        lstack1 = st.tile([32, 256], F32, tag="lstack1")
        av1 = []
        for j in range(NSLOT):
            kva = dyn.tile([96, KV[j]], BF16, tag="kvTa", bufs=2)
            kvb = dyn.tile([97, KV[j]], BF16, tag="kvTb", bufs=2)
            nc.sync.dma_start(kva[:], dr[f"kvT{j}"].ap()[0:96, :])
            nc.sync.dma_start(kvb[:], dr[f"kvT{j}"].ap()[96:193, :])
            qta = dyn.tile([96, 256], BF16, tag="qTa")
            qtb = dyn.tile([97, 256], BF16, tag="qTb")
            nc.sync.dma_start(qta[:], dr["qT"].ap()[193 * j:193 * j + 96, :])
            nc.sync.dma_start(qtb[:], dr["qT"].ap()[193 * j + 96:193 * j + 193, :])
            qsrc = [[(C[f"cwq{g}"][0][:], qta[:]), (C[f"cwq{g}"][1][:], qtb[:])]
                    for g in range(2)]
            ksrc = [[(C[f"cwk{g}"][0][:], kva[:]), (C[f"cwk{g}"][1][:], kvb[:])]
                    for g in range(2)]
            def vsrc(i, kva=kva, kvb=kvb):
                return [(kva[:, 128 * i:128 * i + 128], C["cwv"][0][:]),
                        (kvb[:, 128 * i:128 * i + 128], C["cwv"][1][:])]
            av_sb = attn_core(j, qsrc, ksrc, vsrc,
                              (dr[f"msk{j}"].ap()[:], 1), None,
                              KV[j], 25, lstack1, "s1")
            av1.append(av_sb)
        rstack1 = st.tile([32, 256], BF16, tag="rstack1")
        nc.vector.reciprocal(rstack1[:], lstack1[:])
        r1 = []
        for j in range(NSLOT):
            r1.append(attn_finish(j, av1[j], rstack1, C["cwo"], None, "r1"))
        if debug:
            nc.sync.dma_start(dbg["d_avn"].ap()[:], av1[0][0][:])
            nc.sync.dma_start(dbg["d_r1"].ap()[:], r1[0][0][:])

        # ================= stages 2 (row) and 3 (col) =================
        def axial_stage(pre, res, res_tag, sname):
            x_all = [res[f][tt] for f in range(NSLOT) for tt in range(2)]
            aggs, rs = ln_stats(x_all, sname)
            lst = st.tile([32, 256], F32, tag=f"lstack_{sname}")
            avs = []
            for f in range(NSLOT):
                xh = [ln_apply(res[f][tt], aggs, rs, 2 * f + tt, sname)
                      for tt in range(2)]
                xT = transpose_pair(xh, sname)
                if debug and f == 0 and pre == "r":
                    nc.sync.dma_start(dbg["d_xh"].ap()[:], xh[0][:])
                    nc.gpsimd.dma_start(dbg["d_xT"].ap()[:], xT[0][:])
                qsrc = [[(C[f"{pre}wq{ch}{g}"][:], xT[ch][:]) for ch in range(2)]
                        for g in range(2)]
                ksrc = [[(C[f"{pre}wk{ch}{g}"][:], xT[ch][:]) for ch in range(2)]
                        for g in range(2)]
                def vsrc(i, xT=xT):
                    return [(xT[ch][0:97, 128 * i:128 * i + 128],
                             C[f"{pre}wv{ch}"][:]) for ch in range(2)]
                av_sb = attn_core(f, qsrc, ksrc, vsrc,
                                  (C[f"{pre}ka"][:], 3),
                                  C[f"{pre}qa"][:],
                                  NQ, 27, lst, pre)
                avs.append(av_sb)
            rst = st.tile([32, 256], BF16, tag=f"rstack_{sname}")
            nc.vector.reciprocal(rst[:], lst[:])
            return [attn_finish(f, avs[f], rst, C[f"{pre}wo"], res[f], res_tag)
                    for f in range(NSLOT)]

        def dump_partial(res):
            for f in range(NSLOT):
                for tt in range(2):
                    row = 128 * (2 * f + tt)
                    nc.gpsimd.dma_start(out.ap()[row:row + 128, 0:D], res[f][tt][:])

        if stop_stage == 1:
            dump_partial(r1)
            r2 = None
        else:
            r2 = axial_stage("r", r1, "r2", "s2")
        if debug and r2 is not None:
            nc.sync.dma_start(dbg["d_r2"].ap()[:], r2[0][0][:])
        if stop_stage == 2 and r2 is not None:
            dump_partial(r2)
        r3 = axial_stage("l", r2, "r3", "s3") if stop_stage >= 3 else None
        if debug and r3 is not None:
            nc.sync.dma_start(dbg["d_r3"].ap()[:], r3[0][0][:])

        # ================= stage 4: FFN + head =================
        if stop_stage == 3 and r3 is not None:
            dump_partial(r3)
        x_all = [r3[f][tt] for f in range(NSLOT) for tt in range(2)] \
            if stop_stage >= 4 else []
        aggs4, rs4 = ln_stats(x_all, "s4") if stop_stage >= 4 else (None, None)
        for f in range(NSLOT if stop_stage >= 4 else 0):
            xh = [ln_apply(r3[f][tt], aggs4, rs4, 2 * f + tt, "s4")
                  for tt in range(2)]
            xT = transpose_pair(xh, "s4")
            h1g = []
            for q in range(6):
                ph = pg(128, 256)
                for ch in range(2):
                    nc.tensor.matmul(ph[:],
                                     C["fw1"][ch][:, 128 * q:128 * q + 128],
                                     xT[ch][:], start=(ch == 0), stop=(ch == 1))
                hg = dyn.tile([128, 256], BF16, tag=f"h1g{q}")
                nc.scalar.activation(hg[:], ph[:], gelu_f)
                h1g.append(hg)
            if debug and f == 0:
                nc.gpsimd.dma_start(dbg["d_h1g"].ap()[:], h1g[0][:])
            z = []
            for tt in range(2):
                pz = pg(128, D)
                for q in range(6):
                    nc.tensor.matmul(pz[:], h1g[q][:, 128 * tt:128 * tt + 128],
                                     C["fw2"][:, D * q:D * q + D],
                                     start=(q == 0), stop=(q == 5))
                zt = dy3.tile([128, D], F32, tag="z_t")
                nc.vector.tensor_tensor(zt[:], pz[:], C["fb2"][:], OP.add)
                z_sb = st.tile([128, D], BF16, tag=f"z_{f}_{tt}")
                nc.vector.tensor_tensor(z_sb[:], zt[:], r3[f][tt][:], OP.add)
                z.append(z_sb)
            if debug and f == 0:
                nc.sync.dma_start(dbg["d_z"].ap()[:], z[0][:])
            zT = transpose_pair(z, "hz", ones_row=False)
            for tt in range(2):
                po = pg(128, QL)
                for ch in range(2):
                    nc.tensor.matmul(po[:], zT[ch][0:96, 128 * tt:128 * tt + 128],
                                     C["hw"][:, QL * ch:QL * ch + QL],
                                     start=(ch == 0), stop=(ch == 1))
                ot = dy3.tile([128, QL], F32, tag="o_t")
                nc.vector.tensor_tensor(ot[:], po[:], C["hb"][:], OP.add)
                row = 128 * (2 * f + tt)
                nc.gpsimd.dma_start(out.ap()[row:row + 128, :], ot[:])

    nc.compile()
    _CACHE[key] = nc
    return nc


# ---------------------------------------------------------------- entry

def kernel(**inputs):
    inputs = {k: np.asarray(v, np.float32) for k, v in inputs.items()}
    nc = build_program()
    const = _host_constants(inputs)
    in_maps = [_core_inputs(inputs, const, c) for c in range(NCORE)]
    res = bass_utils.run_bass_kernel_spmd(nc, in_maps, core_ids=list(range(NCORE)))
    out = np.zeros((B, T - 1, H, W, QL), np.float32)
    for f in range(30):
        b, t = _frame(f)
        core, j = f % 8, f // 8
        o = res.results[core]["out"].reshape(NSLOT, 2 * 128, QL)
        out[b, t - 1] = o[j].reshape(H, W, QL)
    return out



# revision 38
# speedup vs baseline: 2.6283x; 1.2569x over previous
"""Trainium2 Bass kernel for nn_DecoderVectorized (axial decoder with causal
cross-attention). Self-contained: hardcodes all shapes/sharding.

Sharding: 32 SPMD slots = 8 cores x 4 slots over the B*(T-1)=30 frames
(sorted by t so slot j has kv prefix 128*{4,8,12,15}; 2 dummy slots).
"""
import math
import sys

import numpy as np

try:
    import concourse.bass as bass
except ImportError:  # pragma: no cover
    sys.path.insert(0, "/opt/trn_rl_repo")
    import concourse.bass as bass

import concourse.bacc as bacc
import concourse.mybir as mybir
import concourse.tile as tile
from concourse import bass_utils
from concourse.masks import make_identity

F32 = mybir.dt.float32
F32R = mybir.dt.float32r
BF16 = mybir.dt.bfloat16
import ml_dtypes
NPBF = ml_dtypes.bfloat16
AF = mybir.ActivationFunctionType
OP = mybir.AluOpType

H, W, D, HEADS, QL = 16, 16, 192, 8, 256
B, T, M = 2, 16, 128
NQ = H * W          # 256 tokens per frame
DH = D // HEADS     # 24
NCORE, NSLOT = 8, 4
TMAX = [4, 8, 12, 16]
KV = [t * 128 for t in TMAX]        # 512 1024 1536 2048
CM = 32.0                           # mask bias (power of 2: bf16-exact)
SCL = 1.0 / math.sqrt(DH)
EPS = 1e-5


def _frame(f):
    """frame index f in [0,32) -> (b, t); 30/31 are dummies."""
    if f >= 30:
        return (f - 30, 15)
    return (f % 2, f // 2 + 1)


# ---------------------------------------------------------------- host prep

def _qk_colmat_s1(w, bvec, g, is_q):
    """[193,128] colmat for stage-1 qhT/khT half g. Head h data at cols
    32h+1..32h+24 (aux row 0 first); col 32h+0: e_192 (ones) if is_q."""
    m = np.zeros((193, 128), np.float32)
    for h in range(4):
        Hh = 4 * g + h
        m[0:192, 32 * h + 1:32 * h + 25] = w[:, DH * Hh:DH * Hh + DH]
        m[192, 32 * h + 1:32 * h + 25] = bvec[DH * Hh:DH * Hh + DH]
        if is_q:
            m[192, 32 * h] = 1.0
    return m


def _qk_colmat_23(w, bvec, ch, g):
    """[97,128] colmat chunk for stage-2/3 qhT/khT. Data at cols 32h+0..+23
    (32-aligned partition start for the data matmul); axial-mask aux rows
    are contracted from separate const tiles."""
    m = np.zeros((97, 128), np.float32)
    for h in range(4):
        Hh = 4 * g + h
        m[0:96, 32 * h:32 * h + 24] = w[96 * ch:96 * ch + 96, DH * Hh:DH * Hh + DH]
        m[96, 32 * h:32 * h + 24] = bvec[DH * Hh:DH * Hh + DH] * 0.5
    return m


def _wv_colmat_s1(w, bvec):
    """[193,256]: head Hh data at cols 32Hh..+23, ones col at 32Hh+24."""
    m = np.zeros((193, 256), np.float32)
    for Hh in range(8):
        m[0:192, 32 * Hh:32 * Hh + 24] = w[:, DH * Hh:DH * Hh + DH]
        m[192, 32 * Hh:32 * Hh + 24] = bvec[DH * Hh:DH * Hh + DH]
        m[192, 32 * Hh + 24] = 1.0
    return m


def _wv_colmat_23(w, bvec, ch):
    m = np.zeros((97, 256), np.float32)
    for Hh in range(8):
        m[0:96, 32 * Hh:32 * Hh + 24] = w[96 * ch:96 * ch + 96, DH * Hh:DH * Hh + DH]
        m[96, 32 * Hh:32 * Hh + 24] = bvec[DH * Hh:DH * Hh + DH] * 0.5
        m[96, 32 * Hh + 24] = 0.5
    return m


def _wo_aug(w, bvec):
    """[128, 384]: head H=4g+h at partition rows 32h..32h+31, col block 192g:
    rows 0..23 = wo rows, row 24 = bo/8 (walrus needs lhsT/rhs same base)."""
    m = np.zeros((128, 2 * D), np.float32)
    for g in range(2):
        for h in range(4):
            Hh = 4 * g + h
            m[32 * h:32 * h + 24, D * g:D * g + D] = w[DH * Hh:DH * Hh + DH, :]
            m[32 * h + 24, D * g:D * g + D] = bvec / 8.0
    return m


def _aux_rows(idx, is_q):
    """[128,256] aux contraction rows for the rank-3 axial mask, pre-spread to
    partition rows 32h+0..2. k-side: [ri^2, ri, 1]; q-side: [-c, 2c rj, -c rj^2]."""
    r = idx.astype(np.float32)
    if is_q:
        rows = np.stack([np.full(NQ, -CM, np.float32), 2.0 * CM * r, -CM * r * r])
    else:
        rows = np.stack([r * r, r, np.ones(NQ, np.float32)])
    m = np.zeros((128, NQ), np.float32)
    for h in range(4):
        m[32 * h:32 * h + 3] = rows
    return m


def _host_constants(inp):
    """Shared (core-independent) device constant arrays."""
    c = {}
    g, b_ = inp["rn_g"], inp["rn_b"]

    def eff(wq, bq, scale):
        return (g[:, None] * wq * scale).astype(np.float32), \
               ((b_ @ wq + bq) * scale).astype(np.float32)

    for gg in range(2):
        c[f"cwq{gg}"] = _qk_colmat_s1(inp["c_wq"] * SCL, inp["c_bq"] * SCL, gg, True)
        c[f"cwk{gg}"] = _qk_colmat_s1(inp["c_wk"], inp["c_bk"], gg, False)
    c["cwv"] = _wv_colmat_s1(inp["c_wv"], inp["c_bv"])
    c["cwo"] = _wo_aug(inp["c_wo"], inp["c_bo"])
    tok = np.arange(NQ)
    for pre, wp, idx in (("r", "r", tok // 16), ("l", "col", tok % 16)):
        wq, bq = eff(inp[wp + "_wq"], inp[wp + "_bq"], SCL)
        wk, bk = eff(inp[wp + "_wk"], inp[wp + "_bk"], 1.0)
        wv, bv = eff(inp[wp + "_wv"], inp[wp + "_bv"], 1.0)
        for ch in range(2):
            for gg in range(2):
                c[f"{pre}wq{ch}{gg}"] = _qk_colmat_23(wq, bq, ch, gg)
                c[f"{pre}wk{ch}{gg}"] = _qk_colmat_23(wk, bk, ch, gg)
            c[f"{pre}wv{ch}"] = _wv_colmat_23(wv, bv, ch)
        c[f"{pre}wo"] = _wo_aug(inp[wp + "_wo"], inp[wp + "_bo"])
        c[f"{pre}ka"] = _aux_rows(idx, False)
        c[f"{pre}qa"] = _aux_rows(idx, True)
    w1 = (inp["ff_ln_g"][:, None] * inp["ff_w1"]).astype(np.float32)
    b1 = (inp["ff_ln_b"] @ inp["ff_w1"] + inp["ff_b1"]).astype(np.float32)
    fw1 = np.zeros((2 * 97, 4 * D), np.float32)
    for ch in range(2):
        fw1[97 * ch:97 * ch + 96] = w1[96 * ch:96 * ch + 96]
        fw1[97 * ch + 96] = b1 * 0.5
    c["fw1"] = fw1
    fw2 = np.zeros((128, 6 * D), np.float32)
    for q in range(6):
        fw2[:, D * q:D * q + D] = inp["ff_w2"][128 * q:128 * q + 128, :]
    c["fw2"] = fw2
    c["fb2"] = np.broadcast_to(inp["ff_b2"][None], (128, D)).copy().astype(np.float32)
    hw = np.zeros((96, 2 * QL), np.float32)
    hw[:, 0:QL] = inp["head_w"][0:96]
    hw[:, QL:2 * QL] = inp["head_w"][96:192]
    c["hw"] = hw
    c["hb"] = np.broadcast_to(inp["head_b"][None], (128, QL)).copy().astype(np.float32)
    rp = np.zeros((32, 8 * 128), np.float32)
    for k in range(8):
        for h in range(4):
            rp[4 * k + h, 128 * k + 32 * h:128 * k + 32 * h + 32] = 1.0
    c["repl"] = rp
    # all matmul operands stream as bf16 (1 PE cycle/row vs 4 for fp32)
    for nm in c:
        if nm not in ("fb2", "hb"):
            c[nm] = c[nm].astype(NPBF)
    return c


def _core_inputs(inp, const, core):
    """Per-core in_map (includes the shared consts)."""
    m = dict(const)
    qg = np.asarray(inp["query_grid"], np.float32)
    tp = np.asarray(inp["t_pos_w"], np.float32)
    mt = np.asarray(inp["mem_tokens"], np.float32)
    qT = np.zeros((NSLOT * 193, NQ), np.float32)
    for j in range(NSLOT):
        b, t = _frame(8 * j + core)
        qT[193 * j:193 * j + 192] = (qg + tp[t][None, :]).T
        qT[193 * j + 192] = 1.0
        kvT = np.ones((193, KV[j]), np.float32)
        kvT[0:192] = mt[b, :TMAX[j]].reshape(-1, D).T
        m[f"kvT{j}"] = kvT.astype(NPBF)
        msk = np.zeros((4, KV[j]), np.float32)
        msk[:, 128 * t:] = -CM
        m[f"msk{j}"] = msk.astype(NPBF)
    m["qT"] = qT.astype(NPBF)
    return m


# ---------------------------------------------------------------- program

_CACHE = {}

# consts whose DRAM row-count exceeds 128: load as (rows0:97|0:96, rest) pairs
_SPLIT193 = ("cwq0", "cwq1", "cwk0", "cwk1", "cwv")


def build_program(gelu_f=AF.Gelu, debug=False, stop_stage=4):
    key = (gelu_f, debug, stop_stage)
    if key in _CACHE:
        return _CACHE[key]
    nc = bacc.Bacc("TRN2", target_bir_lowering=False, debug=False)

    # ---- DRAM I/O ----
    dr = {}
    def din(name, shape, dt_=BF16):
        dr[name] = nc.dram_tensor(name, shape, dt_, kind="ExternalInput")
    for gg in range(2):
        din(f"cwq{gg}", (193, 128)); din(f"cwk{gg}", (193, 128))
    din("cwv", (193, 256)); din("cwo", (128, 2 * D))
    for pre in ("r", "l"):
        for ch in range(2):
            for gg in range(2):
                din(f"{pre}wq{ch}{gg}", (97, 128)); din(f"{pre}wk{ch}{gg}", (97, 128))
            din(f"{pre}wv{ch}", (97, 256))
        din(f"{pre}wo", (128, 2 * D))
        din(f"{pre}ka", (128, NQ)); din(f"{pre}qa", (128, NQ))
    din("fw1", (2 * 97, 4 * D)); din("fw2", (128, 6 * D))
    din("fb2", (128, D), F32)
    din("hw", (96, 2 * QL)); din("hb", (128, QL), F32)
    din("repl", (32, 8 * 128))
    din("qT", (NSLOT * 193, NQ))
    for j in range(NSLOT):
        din(f"kvT{j}", (193, KV[j])); din(f"msk{j}", (4, KV[j]))
    out = nc.dram_tensor("out", (NSLOT * 2 * 128, QL), F32, kind="ExternalOutput")
    dbg = {}
    if debug:
        for nm, shape, dt_ in (("d_qhT", (128, 256), BF16), ("d_khT", (128, 512), BF16),
                               ("d_pT", (128, 1024), BF16), ("d_av", (128, 256), F32),
                               ("d_avn", (128, 256), F32), ("d_r1", (128, D), F32),
                               ("d_xh", (128, D), BF16), ("d_xT", (97, 256), BF16),
                               ("d_r2", (128, D), F32), ("d_r3", (128, D), F32),
                               ("d_h1g", (128, 256), BF16), ("d_z", (128, D), BF16)):
            dbg[nm] = nc.dram_tensor(nm, shape, dt_, kind="ExternalOutput")

    from contextlib import ExitStack
    with tile.TileContext(nc) as tc, ExitStack() as es, \
            nc.allow_low_precision(reason="bf16 operands; rel-err gate 2e-2"):
        cst = es.enter_context(tc.tile_pool(name="cst", bufs=1))
        dyn = es.enter_context(tc.tile_pool(name="dyn", bufs=2))
        dy3 = es.enter_context(tc.tile_pool(name="dy3", bufs=3))
        dy8 = es.enter_context(tc.tile_pool(name="dy8", bufs=8))
        st = es.enter_context(tc.tile_pool(name="st", bufs=1))
        ps_s = es.enter_context(tc.tile_pool(name="ps_s", bufs=2, space="PSUM"))
        ps_a = es.enter_context(tc.tile_pool(name="ps_a", bufs=2, space="PSUM"))
        ps_g = es.enter_context(tc.tile_pool(name="ps_g", bufs=2, space="PSUM"))

        def rr(ap):
            return ap.bitcast(mybir.dt.float32r)

        def pg(p_, f_, dt_=F32):
            return ps_g.tile([p_, f_], dt_, tag="pg", name="pg")

        # ---- load constants ----
        C = {}
        for nm, t_ in dr.items():
            if nm == "qT" or nm.startswith(("kvT", "msk")):
                continue
            shape = list(t_.shape)
            dt_ = t_.dtype
            if nm in _SPLIT193:
                ta = cst.tile([96, shape[1]], dt_, tag=nm + "a")
                tb = cst.tile([97, shape[1]], dt_, tag=nm + "b")
                nc.sync.dma_start(ta[:], t_.ap()[0:96, :])
                nc.sync.dma_start(tb[:], t_.ap()[96:193, :])
                C[nm] = (ta, tb)
            elif nm == "fw1":
                ta = cst.tile([97, shape[1]], dt_, tag="fw1a")
                tb = cst.tile([97, shape[1]], dt_, tag="fw1b")
                nc.sync.dma_start(ta[:], t_.ap()[0:97, :])
                nc.sync.dma_start(tb[:], t_.ap()[97:194, :])
                C[nm] = (ta, tb)
            else:
                tl = cst.tile(shape, dt_, tag=nm)
                nc.sync.dma_start(tl[:], t_.ap()[:])
                C[nm] = tl
        ident = cst.tile([128, 128], BF16, tag="ident")
        make_identity(nc, ident[:])
        epsc = cst.tile([128, 1], F32, tag="epsc")
        nc.gpsimd.memset(epsc[:], EPS)

        def evac(dst_ap, src_ap):
            nc.vector.tensor_copy(dst_ap, src_ap)

        # ============ generic attention core ============
        def attn_core(iid, qsrc, ksrc, vsrc, kaux, kvlen, kp, lstack, pres):
            """qsrc/ksrc: per half, list of (lhsT_chunk, rhs_chunk) pairs.
            vsrc(i): chunk pairs for kv-tile i. kaux: s1 -> DRAM msk ap
            (1 bias row/head, strided-DMA'd into khT rows 32h); s2/3 ->
            (ka, qa) const tiles contracted via a separate K=3 matmul.
            Returns av_sb (unnormalized ovT + l rows) per half."""
            nkt = kvlen // 128
            qhT, khT = [], []
            for g in range(2):
                pq = pg(128, 256)
                for i, (cm, rhs) in enumerate(qsrc[g]):
                    nc.tensor.matmul(pq[:], cm, rhs, start=(i == 0),
                                     stop=(i == len(qsrc[g]) - 1))
                q_sb = dyn.tile([128, 256], BF16, tag=f"qhT{g}")
                evac(q_sb[:], pq[:])
                qhT.append(q_sb)
                k_sb = dyn.tile([128, kvlen], BF16, tag=f"khT{g}", bufs=1)
                for nchunk in range(0, kvlen, 512):
                    nw = min(512, kvlen - nchunk)
                    pk = pg(128, 512)
                    for i, (cm, rhs) in enumerate(ksrc[g]):
                        nc.tensor.matmul(pk[:, 0:nw], cm,
                                         rhs[:, nchunk:nchunk + nw],
                                         start=(i == 0),
                                         stop=(i == len(ksrc[g]) - 1))
                    evac(k_sb[:, nchunk:nchunk + nw], pk[:, 0:nw])
                if pres == "s1":          # dynamic mask row -> partitions 32h
                    nc.gpsimd.dma_start(k_sb[0:97:32, :], kaux[:])
                khT.append(k_sb)
            av = [ps_a.tile([128, 256], F32, tag="p_av", name="p_av") for _ in range(2)]
            npair = nkt // 2
            for ip in range(npair):
                vhs = []
                for d in range(2):
                    pv = pg(128, 256)
                    vch = vsrc(2 * ip + d)
                    for ci, (cm, rhs) in enumerate(vch):
                        nc.tensor.matmul(pv[:], cm, rhs, start=(ci == 0),
                                         stop=(ci == len(vch) - 1))
                    vh = dy3.tile([128, 256], BF16, tag="vh")
                    evac(vh[:], pv[:])
                    vhs.append(vh)
                for g in range(2):
                    # two head-pair psum tiles (bufs=2): PE fills one while
                    # Act exps the other. One head per psum bank (cols 512hh)
                    # -- concurrent row-group matmuls to one bank collide
                    pTs = []
                    for hp in range(2):
                        psd = ps_s.tile([128, 1024], F32, tag="p_sT", bufs=2,
                                        name="p_sT")
                        for hh in range(2):
                            h = 2 * hp + hh
                            for d in range(2):
                                sl = psd[:, 512 * hh + 256 * d:
                                         512 * hh + 256 * d + 256]
                                ck = 128 * (2 * ip + d)
                                if pres == "s1":
                                    nc.tensor.matmul(
                                        sl, khT[g][32 * h:32 * h + kp,
                                                   ck:ck + 128],
                                        qhT[g][32 * h:32 * h + kp, :],
                                        start=True, stop=True,
                                        tile_position=(32 * h, 0))
                                else:
                                    nc.tensor.matmul(
                                        sl, khT[g][32 * h:32 * h + 24,
                                                   ck:ck + 128],
                                        qhT[g][32 * h:32 * h + 24, :],
                                        start=True, stop=False,
                                        tile_position=(32 * h, 0))
                                    nc.tensor.matmul(
                                        sl, kaux[0][32 * h:32 * h + 3,
                                                    ck:ck + 128],
                                        kaux[1][32 * h:32 * h + 3, :],
                                        start=False, stop=True,
                                        tile_position=(32 * h, 0))
                        pTd = dy3.tile([128, 1024], BF16, tag="pT", bufs=4)
                        nc.scalar.activation(pTd[:], psd[:], AF.Exp)
                        pTs.append(pTd)
                    if debug and iid == 0 and ip == 0 and g == 0 and pres == "s1":
                        nc.gpsimd.dma_start(dbg["d_pT"].ap()[:], pTs[0][:])
                    for hp in range(2):
                        for hh in range(2):
                            h = 2 * hp + hh
                            for d in range(2):
                                nc.tensor.matmul(
                                    av[g][32 * h:32 * h + 32, :],
                                    vhs[d][:, 32 * (4 * g + h):32 * (4 * g + h) + 32],
                                    pTs[hp][:, 512 * hh + 256 * d:
                                            512 * hh + 256 * d + 256],
                                    start=(ip == 0 and d == 0),
                                    stop=(ip == npair - 1 and d == 1),
                                    tile_position=(0, 32 * h), skip_group_check=True)
            av_sb = []
            for g in range(2):
                a_sb = dy8.tile([128, 256], F32, tag="av_sb")
                evac(a_sb[:], av[g][:])
                # l rows live at partitions 32h+24: strided DMA extracts all 4
                nc.gpsimd.dma_start(lstack[8 * iid + 4 * g:8 * iid + 4 * g + 4, :],
                                    a_sb[24:121:32, :])
                av_sb.append(a_sb)
            if debug and iid == 0 and pres == "s1":
                nc.gpsimd.dma_start(dbg["d_qhT"].ap()[:], qhT[0][:])
                nc.gpsimd.dma_start(dbg["d_khT"].ap()[:], khT[0][:, 0:512])
                nc.sync.dma_start(dbg["d_av"].ap()[:], av_sb[0][:])
            return av_sb

        def attn_finish(iid, av_sb, rstack, wo_t, res_in, res_tag):
            """r-broadcast, normalize, wo projection (+residual)."""
            avn = []
            for g in range(2):
                prb = pg(128, 256)
                k = 2 * iid + g
                nc.tensor.matmul(prb[:], C["repl"][:, 128 * k:128 * k + 128],
                                 rstack[:], start=True, stop=True)
                an = dy3.tile([128, 256], BF16, tag="avn")
                nc.vector.tensor_tensor(an[:], av_sb[g][:], prb[:], OP.mult)
                avn.append(an)
            outs = []
            for tt in range(2):
                py = pg(128, D)
                for g in range(2):
                    nc.tensor.matmul(
                        py[:], avn[g][:, 128 * tt:128 * tt + 128],
                        wo_t[:, D * g:D * g + D],
                        start=(g == 0), stop=(g == 1))
                r_new = st.tile([128, D], F32, tag=f"{res_tag}_{iid}_{tt}")
                if res_in is None:
                    evac(r_new[:], py[:])
                else:
                    nc.vector.tensor_tensor(r_new[:], res_in[tt][:], py[:], OP.add)
                outs.append(r_new)
            return outs

        # ============ LN helpers ============
        def ln_stats(x_tiles, name):
            # tensor_tensor_reduce is fatal at runtime on this stack; use
            # bn_stats/bn_aggr (one DVE pass -> mean, var per partition)
            n = len(x_tiles)
            var = dyn.tile([128, n], F32, tag=f"var_{name}")
            rs = dyn.tile([128, n], F32, tag=f"rs_{name}")
            aggs = []
            for k, xt in enumerate(x_tiles):
                bst = dy3.tile([128, 6], F32, tag="bst")
                nc.vector.bn_stats(bst[:], xt[:])
                agg = dy8.tile([128, 2], F32, tag=f"agg_{name}", name="agg")
                nc.vector.bn_aggr(agg[:], bst[:])
                nc.vector.tensor_copy(var[:, k:k + 1], agg[:, 1:2])
                aggs.append(agg)
            lnv = dyn.tile([128, n], F32, tag=f"lnv_{name}")
            nc.scalar.activation(lnv[:], var[:], AF.Ln, bias=epsc[:])
            nc.scalar.activation(rs[:], lnv[:], AF.Exp, scale=-0.5)
            return aggs, rs

        def ln_apply(x, aggs, rs, k, name):
            xh = dy3.tile([128, D], BF16, tag=f"xh_{name}")
            nc.vector.tensor_scalar(xh[:], x[:], aggs[k][:, 0:1], rs[:, k:k + 1],
                                    OP.subtract, OP.mult)
            return xh

        def transpose_pair(xh_tiles, name, ones_row=True):
            xT = []
            for ch in range(2):
                t_ = dyn.tile([97, 256], BF16, tag=f"xT{ch}", name=f"xT{ch}")
                for tt in range(2):
                    pt = pg(96, 128, BF16)
                    nc.tensor.transpose(pt[:], xh_tiles[tt][:, 96 * ch:96 * ch + 96],
                                        ident[:])
                    evac(t_[0:96, 128 * tt:128 * tt + 128], pt[:])
                if ones_row:
                    nc.gpsimd.memset(t_[96:97, :], 1.0)
                xT.append(t_)
            return xT

        # ================= stage 1: cross attention =================
        lstack1 = st.tile([32, 256], F32, tag="lstack1")
        av1 = []
        for j in range(NSLOT):
            kva = dyn.tile([96, KV[j]], BF16, tag="kvTa", bufs=2)
            kvb = dyn.tile([97, KV[j]], BF16, tag="kvTb", bufs=2)
            nc.sync.dma_start(kva[:], dr[f"kvT{j}"].ap()[0:96, :])
            nc.sync.dma_start(kvb[:], dr[f"kvT{j}"].ap()[96:193, :])
            qta = dyn.tile([96, 256], BF16, tag="qTa")
            qtb = dyn.tile([97, 256], BF16, tag="qTb")
            nc.sync.dma_start(qta[:], dr["qT"].ap()[193 * j:193 * j + 96, :])
            nc.sync.dma_start(qtb[:], dr["qT"].ap()[193 * j + 96:193 * j + 193, :])
            qsrc = [[(C[f"cwq{g}"][0][:], qta[:]), (C[f"cwq{g}"][1][:], qtb[:])]
                    for g in range(2)]
            ksrc = [[(C[f"cwk{g}"][0][:], kva[:]), (C[f"cwk{g}"][1][:], kvb[:])]
                    for g in range(2)]
            def vsrc(i, kva=kva, kvb=kvb):
                return [(kva[:, 128 * i:128 * i + 128], C["cwv"][0][:]),
                        (kvb[:, 128 * i:128 * i + 128], C["cwv"][1][:])]
            av_sb = attn_core(j, qsrc, ksrc, vsrc, dr[f"msk{j}"].ap()[:],
                              KV[j], 25, lstack1, "s1")
            av1.append(av_sb)
        rstack1 = st.tile([32, 256], BF16, tag="rstack1")
        nc.vector.reciprocal(rstack1[:], lstack1[:])
        r1 = []
        for j in range(NSLOT):
            r1.append(attn_finish(j, av1[j], rstack1, C["cwo"], None, "r1"))
        if debug:
            nc.sync.dma_start(dbg["d_avn"].ap()[:], av1[0][0][:])
            nc.sync.dma_start(dbg["d_r1"].ap()[:], r1[0][0][:])

        # ================= stages 2 (row) and 3 (col) =================
        def axial_stage(pre, res, res_tag, sname):
            x_all = [res[f][tt] for f in range(NSLOT) for tt in range(2)]
            aggs, rs = ln_stats(x_all, sname)
            lst = st.tile([32, 256], F32, tag=f"lstack_{sname}")
            avs = []
            for f in range(NSLOT):
                xh = [ln_apply(res[f][tt], aggs, rs, 2 * f + tt, sname)
                      for tt in range(2)]
                xT = transpose_pair(xh, sname)
                if debug and f == 0 and pre == "r":
                    nc.sync.dma_start(dbg["d_xh"].ap()[:], xh[0][:])
                    nc.gpsimd.dma_start(dbg["d_xT"].ap()[:], xT[0][:])
                qsrc = [[(C[f"{pre}wq{ch}{g}"][:], xT[ch][:]) for ch in range(2)]
                        for g in range(2)]
                ksrc = [[(C[f"{pre}wk{ch}{g}"][:], xT[ch][:]) for ch in range(2)]
                        for g in range(2)]
                def vsrc(i, xT=xT):
                    return [(xT[ch][0:97, 128 * i:128 * i + 128],
                             C[f"{pre}wv{ch}"][:]) for ch in range(2)]
                av_sb = attn_core(f, qsrc, ksrc, vsrc,
                                  (C[f"{pre}ka"][:], C[f"{pre}qa"][:]),
                                  NQ, 27, lst, pre)
                avs.append(av_sb)
            rst = st.tile([32, 256], BF16, tag=f"rstack_{sname}")
            nc.vector.reciprocal(rst[:], lst[:])
            return [attn_finish(f, avs[f], rst, C[f"{pre}wo"], res[f], res_tag)
                    for f in range(NSLOT)]

        def dump_partial(res):
            for f in range(NSLOT):
                for tt in range(2):
                    row = 128 * (2 * f + tt)
                    nc.gpsimd.dma_start(out.ap()[row:row + 128, 0:D], res[f][tt][:])

        if stop_stage == 1:
            dump_partial(r1)
            r2 = None
        else:
            r2 = axial_stage("r", r1, "r2", "s2")
        if debug and r2 is not None:
            nc.sync.dma_start(dbg["d_r2"].ap()[:], r2[0][0][:])
        if stop_stage == 2 and r2 is not None:
            dump_partial(r2)
        r3 = axial_stage("l", r2, "r3", "s3") if stop_stage >= 3 else None
        if debug and r3 is not None:
            nc.sync.dma_start(dbg["d_r3"].ap()[:], r3[0][0][:])

        # ================= stage 4: FFN + head =================
        if stop_stage == 3 and r3 is not None:
            dump_partial(r3)
        x_all = [r3[f][tt] for f in range(NSLOT) for tt in range(2)] \
            if stop_stage >= 4 else []
        aggs4, rs4 = ln_stats(x_all, "s4") if stop_stage >= 4 else (None, None)
        for f in range(NSLOT if stop_stage >= 4 else 0):
            xh = [ln_apply(r3[f][tt], aggs4, rs4, 2 * f + tt, "s4")
                  for tt in range(2)]
            xT = transpose_pair(xh, "s4")
            h1g = []
            for q in range(6):
                ph = pg(128, 256)
                for ch in range(2):
                    nc.tensor.matmul(ph[:],
                                     C["fw1"][ch][:, 128 * q:128 * q + 128],
                                     xT[ch][:], start=(ch == 0), stop=(ch == 1))
                hg = dyn.tile([128, 256], BF16, tag=f"h1g{q}")
                nc.scalar.activation(hg[:], ph[:], gelu_f)
                h1g.append(hg)
            if debug and f == 0:
                nc.gpsimd.dma_start(dbg["d_h1g"].ap()[:], h1g[0][:])
            z = []
            for tt in range(2):
                pz = pg(128, D)
                for q in range(6):
                    nc.tensor.matmul(pz[:], h1g[q][:, 128 * tt:128 * tt + 128],
                                     C["fw2"][:, D * q:D * q + D],
                                     start=(q == 0), stop=(q == 5))
                zt = dy3.tile([128, D], F32, tag="z_t")
                nc.vector.tensor_tensor(zt[:], pz[:], C["fb2"][:], OP.add)
                z_sb = st.tile([128, D], BF16, tag=f"z_{f}_{tt}")
                nc.vector.tensor_tensor(z_sb[:], zt[:], r3[f][tt][:], OP.add)
                z.append(z_sb)
            if debug and f == 0:
                nc.sync.dma_start(dbg["d_z"].ap()[:], z[0][:])
            zT = transpose_pair(z, "hz", ones_row=False)
            for tt in range(2):
                po = pg(128, QL)
                for ch in range(2):
                    nc.tensor.matmul(po[:], zT[ch][0:96, 128 * tt:128 * tt + 128],
                                     C["hw"][:, QL * ch:QL * ch + QL],
                                     start=(ch == 0), stop=(ch == 1))
                ot = dy3.tile([128, QL], F32, tag="o_t")
                nc.vector.tensor_tensor(ot[:], po[:], C["hb"][:], OP.add)
                row = 128 * (2 * f + tt)
                nc.gpsimd.dma_start(out.ap()[row:row + 128, :], ot[:])

    nc.compile()
    _CACHE[key] = nc
    return nc


# ---------------------------------------------------------------- entry

def kernel(**inputs):
    inputs = {k: np.asarray(v, np.float32) for k, v in inputs.items()}
    nc = build_program()
    const = _host_constants(inputs)
    in_maps = [_core_inputs(inputs, const, c) for c in range(NCORE)]
    res = bass_utils.run_bass_kernel_spmd(nc, in_maps, core_ids=list(range(NCORE)))
    out = np.zeros((B, T - 1, H, W, QL), np.float32)
    for f in range(30):
        b, t = _frame(f)
        core, j = f % 8, f // 8
        o = res.results[core]["out"].reshape(NSLOT, 2 * 128, QL)
        out[b, t - 1] = o[j].reshape(H, W, QL)
    return out



# revision 39
# speedup vs baseline: 2.6971x; 1.0262x over previous
"""Trainium2 Bass kernel for nn_DecoderVectorized (axial decoder with causal
cross-attention). Self-contained: hardcodes all shapes/sharding.

Sharding: 32 SPMD slots = 8 cores x 4 slots over the B*(T-1)=30 frames
(sorted by t so slot j has kv prefix 128*{4,8,12,15}; 2 dummy slots).
"""
import math
import sys

import numpy as np

try:
    import concourse.bass as bass
except ImportError:  # pragma: no cover
    sys.path.insert(0, "/opt/trn_rl_repo")
    import concourse.bass as bass

import concourse.bacc as bacc
import concourse.mybir as mybir
import concourse.tile as tile
from concourse import bass_utils
from concourse.masks import make_identity

F32 = mybir.dt.float32
F32R = mybir.dt.float32r
BF16 = mybir.dt.bfloat16
import ml_dtypes
NPBF = ml_dtypes.bfloat16
AF = mybir.ActivationFunctionType
OP = mybir.AluOpType

H, W, D, HEADS, QL = 16, 16, 192, 8, 256
B, T, M = 2, 16, 128
NQ = H * W          # 256 tokens per frame
DH = D // HEADS     # 24
NCORE, NSLOT = 8, 4
TMAX = [4, 8, 12, 16]
KV = [t * 128 for t in TMAX]        # 512 1024 1536 2048
CM = 32.0                           # mask bias (power of 2: bf16-exact)
SCL = 1.0 / math.sqrt(DH)
EPS = 1e-5


def _frame(f):
    """frame index f in [0,32) -> (b, t); 30/31 are dummies."""
    if f >= 30:
        return (f - 30, 15)
    return (f % 2, f // 2 + 1)


# ---------------------------------------------------------------- host prep

def _qk_colmat_s1(w, bvec, g, is_q):
    """[193,128] colmat for stage-1 qhT/khT half g. Head h data at cols
    32h+1..32h+24 (aux row 0 first); col 32h+0: e_192 (ones) if is_q."""
    m = np.zeros((193, 128), np.float32)
    for h in range(4):
        Hh = 4 * g + h
        m[0:192, 32 * h + 1:32 * h + 25] = w[:, DH * Hh:DH * Hh + DH]
        m[192, 32 * h + 1:32 * h + 25] = bvec[DH * Hh:DH * Hh + DH]
        if is_q:
            m[192, 32 * h] = 1.0
    return m


def _qk_colmat_23(w, bvec, ch, g):
    """[97,128] colmat chunk for stage-2/3 qhT/khT. Data at cols 32h+0..+23
    (32-aligned partition start for the data matmul); axial-mask aux rows
    are contracted from separate const tiles."""
    m = np.zeros((97, 128), np.float32)
    for h in range(4):
        Hh = 4 * g + h
        m[0:96, 32 * h:32 * h + 24] = w[96 * ch:96 * ch + 96, DH * Hh:DH * Hh + DH]
        m[96, 32 * h:32 * h + 24] = bvec[DH * Hh:DH * Hh + DH] * 0.5
    return m


def _wv_colmat_s1(w, bvec):
    """[193,256]: head Hh data at cols 32Hh..+23, ones col at 32Hh+24."""
    m = np.zeros((193, 256), np.float32)
    for Hh in range(8):
        m[0:192, 32 * Hh:32 * Hh + 24] = w[:, DH * Hh:DH * Hh + DH]
        m[192, 32 * Hh:32 * Hh + 24] = bvec[DH * Hh:DH * Hh + DH]
        m[192, 32 * Hh + 24] = 1.0
    return m


def _wv_colmat_23(w, bvec, ch):
    m = np.zeros((97, 256), np.float32)
    for Hh in range(8):
        m[0:96, 32 * Hh:32 * Hh + 24] = w[96 * ch:96 * ch + 96, DH * Hh:DH * Hh + DH]
        m[96, 32 * Hh:32 * Hh + 24] = bvec[DH * Hh:DH * Hh + DH] * 0.5
        m[96, 32 * Hh + 24] = 0.5
    return m


def _wo_aug(w, bvec):
    """[128, 384]: head H=4g+h at partition rows 32h..32h+31, col block 192g:
    rows 0..23 = wo rows, row 24 = bo/8 (walrus needs lhsT/rhs same base)."""
    m = np.zeros((128, 2 * D), np.float32)
    for g in range(2):
        for h in range(4):
            Hh = 4 * g + h
            m[32 * h:32 * h + 24, D * g:D * g + D] = w[DH * Hh:DH * Hh + DH, :]
            m[32 * h + 24, D * g:D * g + D] = bvec / 8.0
    return m


def _aux_rows(idx, is_q):
    """[128,256] aux contraction rows for the rank-3 axial mask, pre-spread to
    partition rows 32h+0..2. k-side: [ri^2, ri, 1]; q-side: [-c, 2c rj, -c rj^2]."""
    r = idx.astype(np.float32)
    if is_q:
        rows = np.stack([np.full(NQ, -CM, np.float32), 2.0 * CM * r, -CM * r * r])
    else:
        rows = np.stack([r * r, r, np.ones(NQ, np.float32)])
    m = np.zeros((128, NQ), np.float32)
    for h in range(4):
        m[32 * h:32 * h + 3] = rows
    return m


def _host_constants(inp):
    """Shared (core-independent) device constant arrays."""
    c = {}
    g, b_ = inp["rn_g"], inp["rn_b"]

    def eff(wq, bq, scale):
        return (g[:, None] * wq * scale).astype(np.float32), \
               ((b_ @ wq + bq) * scale).astype(np.float32)

    for gg in range(2):
        c[f"cwq{gg}"] = _qk_colmat_s1(inp["c_wq"] * SCL, inp["c_bq"] * SCL, gg, True)
        c[f"cwk{gg}"] = _qk_colmat_s1(inp["c_wk"], inp["c_bk"], gg, False)
    c["cwv"] = _wv_colmat_s1(inp["c_wv"], inp["c_bv"])
    c["cwo"] = _wo_aug(inp["c_wo"], inp["c_bo"])
    tok = np.arange(NQ)
    for pre, wp, idx in (("r", "r", tok // 16), ("l", "col", tok % 16)):
        wq, bq = eff(inp[wp + "_wq"], inp[wp + "_bq"], SCL)
        wk, bk = eff(inp[wp + "_wk"], inp[wp + "_bk"], 1.0)
        wv, bv = eff(inp[wp + "_wv"], inp[wp + "_bv"], 1.0)
        for ch in range(2):
            for gg in range(2):
                c[f"{pre}wq{ch}{gg}"] = _qk_colmat_23(wq, bq, ch, gg)
                c[f"{pre}wk{ch}{gg}"] = _qk_colmat_23(wk, bk, ch, gg)
            c[f"{pre}wv{ch}"] = _wv_colmat_23(wv, bv, ch)
        c[f"{pre}wo"] = _wo_aug(inp[wp + "_wo"], inp[wp + "_bo"])
        c[f"{pre}ka"] = _aux_rows(idx, False)
        c[f"{pre}qa"] = _aux_rows(idx, True)
    w1 = (inp["ff_ln_g"][:, None] * inp["ff_w1"]).astype(np.float32)
    b1 = (inp["ff_ln_b"] @ inp["ff_w1"] + inp["ff_b1"]).astype(np.float32)
    fw1 = np.zeros((2 * 97, 4 * D), np.float32)
    for ch in range(2):
        fw1[97 * ch:97 * ch + 96] = w1[96 * ch:96 * ch + 96]
        fw1[97 * ch + 96] = b1 * 0.5
    c["fw1"] = fw1
    fw2 = np.zeros((128, 6 * D), np.float32)
    for q in range(6):
        fw2[:, D * q:D * q + D] = inp["ff_w2"][128 * q:128 * q + 128, :]
    c["fw2"] = fw2
    c["fb2"] = np.broadcast_to(inp["ff_b2"][None], (128, D)).copy().astype(np.float32)
    hw = np.zeros((96, 2 * QL), np.float32)
    hw[:, 0:QL] = inp["head_w"][0:96]
    hw[:, QL:2 * QL] = inp["head_w"][96:192]
    c["hw"] = hw
    c["hb"] = np.broadcast_to(inp["head_b"][None], (128, QL)).copy().astype(np.float32)
    rp = np.zeros((32, 8 * 128), np.float32)
    for k in range(8):
        for h in range(4):
            rp[4 * k + h, 128 * k + 32 * h:128 * k + 32 * h + 32] = 1.0
    c["repl"] = rp
    # all matmul operands stream as bf16 (1 PE cycle/row vs 4 for fp32)
    for nm in c:
        if nm not in ("fb2", "hb"):
            c[nm] = c[nm].astype(NPBF)
    return c


def _core_inputs(inp, const, core):
    """Per-core in_map (includes the shared consts)."""
    m = dict(const)
    qg = np.asarray(inp["query_grid"], np.float32)
    tp = np.asarray(inp["t_pos_w"], np.float32)
    mt = np.asarray(inp["mem_tokens"], np.float32)
    qT = np.zeros((NSLOT * 193, NQ), np.float32)
    for j in range(NSLOT):
        b, t = _frame(8 * j + core)
        qT[193 * j:193 * j + 192] = (qg + tp[t][None, :]).T
        qT[193 * j + 192] = 1.0
        kvT = np.ones((193, KV[j]), np.float32)
        kvT[0:192] = mt[b, :TMAX[j]].reshape(-1, D).T
        m[f"kvT{j}"] = kvT.astype(NPBF)
        msk = np.zeros((4, KV[j]), np.float32)
        msk[:, 128 * t:] = -CM
        m[f"msk{j}"] = msk.astype(NPBF)
    m["qT"] = qT.astype(NPBF)
    return m


# ---------------------------------------------------------------- program

_CACHE = {}

# consts whose DRAM row-count exceeds 128: load as (rows0:97|0:96, rest) pairs
_SPLIT193 = ("cwq0", "cwq1", "cwk0", "cwk1", "cwv")


def build_program(gelu_f=AF.Gelu, debug=False, stop_stage=4):
    key = (gelu_f, debug, stop_stage)
    if key in _CACHE:
        return _CACHE[key]
    nc = bacc.Bacc("TRN2", target_bir_lowering=False, debug=False)

    # ---- DRAM I/O ----
    dr = {}
    def din(name, shape, dt_=BF16):
        dr[name] = nc.dram_tensor(name, shape, dt_, kind="ExternalInput")
    for gg in range(2):
        din(f"cwq{gg}", (193, 128)); din(f"cwk{gg}", (193, 128))
    din("cwv", (193, 256)); din("cwo", (128, 2 * D))
    for pre in ("r", "l"):
        for ch in range(2):
            for gg in range(2):
                din(f"{pre}wq{ch}{gg}", (97, 128)); din(f"{pre}wk{ch}{gg}", (97, 128))
            din(f"{pre}wv{ch}", (97, 256))
        din(f"{pre}wo", (128, 2 * D))
        din(f"{pre}ka", (128, NQ)); din(f"{pre}qa", (128, NQ))
    din("fw1", (2 * 97, 4 * D)); din("fw2", (128, 6 * D))
    din("fb2", (128, D), F32)
    din("hw", (96, 2 * QL)); din("hb", (128, QL), F32)
    din("repl", (32, 8 * 128))
    din("qT", (NSLOT * 193, NQ))
    for j in range(NSLOT):
        din(f"kvT{j}", (193, KV[j])); din(f"msk{j}", (4, KV[j]))
    out = nc.dram_tensor("out", (NSLOT * 2 * 128, QL), F32, kind="ExternalOutput")
    dbg = {}
    if debug:
        for nm, shape, dt_ in (("d_qhT", (128, 256), BF16), ("d_khT", (128, 512), BF16),
                               ("d_pT", (128, 1024), BF16), ("d_av", (128, 256), F32),
                               ("d_avn", (128, 256), F32), ("d_r1", (128, D), F32),
                               ("d_xh", (128, D), BF16), ("d_xT", (97, 256), BF16),
                               ("d_r2", (128, D), F32), ("d_r3", (128, D), F32),
                               ("d_h1g", (128, 256), BF16), ("d_z", (128, D), BF16)):
            dbg[nm] = nc.dram_tensor(nm, shape, dt_, kind="ExternalOutput")

    from contextlib import ExitStack
    with tile.TileContext(nc) as tc, ExitStack() as es, \
            nc.allow_low_precision(reason="bf16 operands; rel-err gate 2e-2"):
        cst = es.enter_context(tc.tile_pool(name="cst", bufs=1))
        dyn = es.enter_context(tc.tile_pool(name="dyn", bufs=2))
        dy3 = es.enter_context(tc.tile_pool(name="dy3", bufs=3))
        dy8 = es.enter_context(tc.tile_pool(name="dy8", bufs=8))
        st = es.enter_context(tc.tile_pool(name="st", bufs=1))
        ps_s = es.enter_context(tc.tile_pool(name="ps_s", bufs=2, space="PSUM"))
        ps_a = es.enter_context(tc.tile_pool(name="ps_a", bufs=2, space="PSUM"))
        ps_g = es.enter_context(tc.tile_pool(name="ps_g", bufs=2, space="PSUM"))

        def rr(ap):
            return ap.bitcast(mybir.dt.float32r)

        def pg(p_, f_, dt_=F32):
            return ps_g.tile([p_, f_], dt_, tag="pg", name="pg")

        # ---- load constants ----
        C = {}
        for nm, t_ in dr.items():
            if nm == "qT" or nm.startswith(("kvT", "msk")):
                continue
            shape = list(t_.shape)
            dt_ = t_.dtype
            if nm in _SPLIT193:
                ta = cst.tile([96, shape[1]], dt_, tag=nm + "a")
                tb = cst.tile([97, shape[1]], dt_, tag=nm + "b")
                nc.sync.dma_start(ta[:], t_.ap()[0:96, :])
                nc.sync.dma_start(tb[:], t_.ap()[96:193, :])
                C[nm] = (ta, tb)
            elif nm == "fw1":
                ta = cst.tile([97, shape[1]], dt_, tag="fw1a")
                tb = cst.tile([97, shape[1]], dt_, tag="fw1b")
                nc.sync.dma_start(ta[:], t_.ap()[0:97, :])
                nc.sync.dma_start(tb[:], t_.ap()[97:194, :])
                C[nm] = (ta, tb)
            else:
                tl = cst.tile(shape, dt_, tag=nm)
                nc.sync.dma_start(tl[:], t_.ap()[:])
                C[nm] = tl
        ident = cst.tile([128, 128], BF16, tag="ident")
        make_identity(nc, ident[:])
        epsc = cst.tile([128, 1], F32, tag="epsc")
        nc.gpsimd.memset(epsc[:], EPS)

        def evac(dst_ap, src_ap):
            nc.vector.tensor_copy(dst_ap, src_ap)

        # ============ generic attention core ============
        def attn_core(iid, qsrc, ksrc, vsrc, kaux, kvlen, kp, lstack, pres):
            """qsrc/ksrc: per half, list of (lhsT_chunk, rhs_chunk) pairs.
            vsrc(i): chunk pairs for kv-tile i. kaux: s1 -> DRAM msk ap
            (1 bias row/head, strided-DMA'd into khT rows 32h); s2/3 ->
            (ka, qa) const tiles contracted via a separate K=3 matmul.
            Returns av_sb (unnormalized ovT + l rows) per half."""
            nkt = kvlen // 128
            qhT, khT = [], []
            for g in range(2):
                pq = pg(128, 256)
                for i, (cm, rhs) in enumerate(qsrc[g]):
                    nc.tensor.matmul(pq[:], cm, rhs, start=(i == 0),
                                     stop=(i == len(qsrc[g]) - 1))
                q_sb = dyn.tile([128, 256], BF16, tag=f"qhT{g}")
                evac(q_sb[:], pq[:])
                qhT.append(q_sb)
                k_sb = dyn.tile([128, kvlen], BF16, tag=f"khT{g}", bufs=1)
                for nchunk in range(0, kvlen, 512):
                    nw = min(512, kvlen - nchunk)
                    pk = pg(128, 512)
                    for i, (cm, rhs) in enumerate(ksrc[g]):
                        nc.tensor.matmul(pk[:, 0:nw], cm,
                                         rhs[:, nchunk:nchunk + nw],
                                         start=(i == 0),
                                         stop=(i == len(ksrc[g]) - 1))
                    evac(k_sb[:, nchunk:nchunk + nw], pk[:, 0:nw])
                if pres == "s1":          # dynamic mask row -> partitions 32h
                    nc.gpsimd.dma_start(k_sb[0:97:32, :], kaux[:])
                khT.append(k_sb)
            av = [ps_a.tile([128, 256], F32, tag="p_av", name="p_av") for _ in range(2)]
            npair = nkt // 2

            def emit_av(ip, g, vhs, pTs):
                for hp in range(2):
                    for hh in range(2):
                        h = 2 * hp + hh
                        for d in range(2):
                            nc.tensor.matmul(
                                av[g][32 * h:32 * h + 32, :],
                                vhs[d][:, 32 * (4 * g + h):32 * (4 * g + h) + 32],
                                pTs[hp][:, 512 * hh + 256 * d:
                                        512 * hh + 256 * d + 256],
                                start=(ip == 0 and d == 0),
                                stop=(ip == npair - 1 and d == 1),
                                tile_position=(0, 32 * h), skip_group_check=True)

            pend = None   # AV mms run one (ip,g)-unit late so the PE stream
            vhs = None    # always has exp-independent work in front of them
            for ip in range(npair):
                for g in range(2):
                    if g == 0:
                        vhs = []
                        for d in range(2):
                            pv = pg(128, 256)
                            vch = vsrc(2 * ip + d)
                            for ci, (cm, rhs) in enumerate(vch):
                                nc.tensor.matmul(pv[:], cm, rhs, start=(ci == 0),
                                                 stop=(ci == len(vch) - 1))
                            vh = dy3.tile([128, 256], BF16, tag="vh", bufs=4)
                            evac(vh[:], pv[:])
                            vhs.append(vh)
                    # two head-pair psum tiles (bufs=2): PE fills one while
                    # Act exps the other. One head per psum bank (cols 512hh)
                    # -- concurrent row-group matmuls to one bank collide
                    pTs = []
                    for hp in range(2):
                        psd = ps_s.tile([128, 1024], F32, tag="p_sT", bufs=2,
                                        name="p_sT")
                        for hh in range(2):
                            h = 2 * hp + hh
                            for d in range(2):
                                sl = psd[:, 512 * hh + 256 * d:
                                         512 * hh + 256 * d + 256]
                                ck = 128 * (2 * ip + d)
                                if pres == "s1":
                                    nc.tensor.matmul(
                                        sl, khT[g][32 * h:32 * h + kp,
                                                   ck:ck + 128],
                                        qhT[g][32 * h:32 * h + kp, :],
                                        start=True, stop=True,
                                        tile_position=(32 * h, 0))
                                else:
                                    nc.tensor.matmul(
                                        sl, khT[g][32 * h:32 * h + 24,
                                                   ck:ck + 128],
                                        qhT[g][32 * h:32 * h + 24, :],
                                        start=True, stop=False,
                                        tile_position=(32 * h, 0))
                                    nc.tensor.matmul(
                                        sl, kaux[0][32 * h:32 * h + 3,
                                                    ck:ck + 128],
                                        kaux[1][32 * h:32 * h + 3, :],
                                        start=False, stop=True,
                                        tile_position=(32 * h, 0))
                        pTd = dy3.tile([128, 1024], BF16, tag="pT", bufs=4)
                        nc.scalar.activation(pTd[:], psd[:], AF.Exp)
                        pTs.append(pTd)
                    if debug and iid == 0 and ip == 0 and g == 0 and pres == "s1":
                        nc.gpsimd.dma_start(dbg["d_pT"].ap()[:], pTs[0][:])
                    if pend is not None:
                        emit_av(*pend)
                    pend = (ip, g, list(vhs), pTs)
            emit_av(*pend)
            av_sb = []
            for g in range(2):
                a_sb = dy8.tile([128, 256], F32, tag="av_sb")
                evac(a_sb[:], av[g][:])
                # l rows live at partitions 32h+24: strided DMA extracts all 4
                nc.gpsimd.dma_start(lstack[8 * iid + 4 * g:8 * iid + 4 * g + 4, :],
                                    a_sb[24:121:32, :])
                av_sb.append(a_sb)
            if debug and iid == 0 and pres == "s1":
                nc.gpsimd.dma_start(dbg["d_qhT"].ap()[:], qhT[0][:])
                nc.gpsimd.dma_start(dbg["d_khT"].ap()[:], khT[0][:, 0:512])
                nc.sync.dma_start(dbg["d_av"].ap()[:], av_sb[0][:])
            return av_sb

        def attn_finish(iid, av_sb, rstack, wo_t, res_in, res_tag):
            """r-broadcast, normalize, wo projection (+residual)."""
            avn = []
            for g in range(2):
                prb = pg(128, 256)
                k = 2 * iid + g
                nc.tensor.matmul(prb[:], C["repl"][:, 128 * k:128 * k + 128],
                                 rstack[:], start=True, stop=True)
                an = dy3.tile([128, 256], BF16, tag="avn")
                nc.vector.tensor_tensor(an[:], av_sb[g][:], prb[:], OP.mult)
                avn.append(an)
            outs = []
            for tt in range(2):
                py = pg(128, D)
                for g in range(2):
                    nc.tensor.matmul(
                        py[:], avn[g][:, 128 * tt:128 * tt + 128],
                        wo_t[:, D * g:D * g + D],
                        start=(g == 0), stop=(g == 1))
                r_new = st.tile([128, D], F32, tag=f"{res_tag}_{iid}_{tt}")
                if res_in is None:
                    evac(r_new[:], py[:])
                else:
                    nc.vector.tensor_tensor(r_new[:], res_in[tt][:], py[:], OP.add)
                outs.append(r_new)
            return outs

        # ============ LN helpers ============
        def ln_stats(x_tiles, name):
            # tensor_tensor_reduce is fatal at runtime on this stack; use
            # bn_stats/bn_aggr (one DVE pass -> mean, var per partition)
            n = len(x_tiles)
            var = dyn.tile([128, n], F32, tag=f"var_{name}")
            rs = dyn.tile([128, n], F32, tag=f"rs_{name}")
            aggs = []
            for k, xt in enumerate(x_tiles):
                bst = dy3.tile([128, 6], F32, tag="bst")
                nc.vector.bn_stats(bst[:], xt[:])
                agg = dy8.tile([128, 2], F32, tag=f"agg_{name}", name="agg")
                nc.vector.bn_aggr(agg[:], bst[:])
                nc.vector.tensor_copy(var[:, k:k + 1], agg[:, 1:2])
                aggs.append(agg)
            lnv = dyn.tile([128, n], F32, tag=f"lnv_{name}")
            nc.scalar.activation(lnv[:], var[:], AF.Ln, bias=epsc[:])
            nc.scalar.activation(rs[:], lnv[:], AF.Exp, scale=-0.5)
            return aggs, rs

        def ln_apply(x, aggs, rs, k, name):
            xh = dy3.tile([128, D], BF16, tag=f"xh_{name}")
            nc.vector.tensor_scalar(xh[:], x[:], aggs[k][:, 0:1], rs[:, k:k + 1],
                                    OP.subtract, OP.mult)
            return xh

        def transpose_pair(xh_tiles, name, ones_row=True):
            xT = []
            for ch in range(2):
                t_ = dyn.tile([97, 256], BF16, tag=f"xT{ch}", name=f"xT{ch}")
                for tt in range(2):
                    pt = pg(96, 128, BF16)
                    nc.tensor.transpose(pt[:], xh_tiles[tt][:, 96 * ch:96 * ch + 96],
                                        ident[:])
                    evac(t_[0:96, 128 * tt:128 * tt + 128], pt[:])
                if ones_row:
                    nc.gpsimd.memset(t_[96:97, :], 1.0)
                xT.append(t_)
            return xT

        # ================= stage 1: cross attention =================
        lstack1 = st.tile([32, 256], F32, tag="lstack1")
        av1 = []
        for j in range(NSLOT):
            kva = dyn.tile([96, KV[j]], BF16, tag="kvTa", bufs=2)
            kvb = dyn.tile([97, KV[j]], BF16, tag="kvTb", bufs=2)
            nc.sync.dma_start(kva[:], dr[f"kvT{j}"].ap()[0:96, :])
            nc.sync.dma_start(kvb[:], dr[f"kvT{j}"].ap()[96:193, :])
            qta = dyn.tile([96, 256], BF16, tag="qTa")
            qtb = dyn.tile([97, 256], BF16, tag="qTb")
            nc.sync.dma_start(qta[:], dr["qT"].ap()[193 * j:193 * j + 96, :])
            nc.sync.dma_start(qtb[:], dr["qT"].ap()[193 * j + 96:193 * j + 193, :])
            qsrc = [[(C[f"cwq{g}"][0][:], qta[:]), (C[f"cwq{g}"][1][:], qtb[:])]
                    for g in range(2)]
            ksrc = [[(C[f"cwk{g}"][0][:], kva[:]), (C[f"cwk{g}"][1][:], kvb[:])]
                    for g in range(2)]
            def vsrc(i, kva=kva, kvb=kvb):
                return [(kva[:, 128 * i:128 * i + 128], C["cwv"][0][:]),
                        (kvb[:, 128 * i:128 * i + 128], C["cwv"][1][:])]
            av_sb = attn_core(j, qsrc, ksrc, vsrc, dr[f"msk{j}"].ap()[:],
                              KV[j], 25, lstack1, "s1")
            av1.append(av_sb)
        rstack1 = st.tile([32, 256], BF16, tag="rstack1")
        nc.vector.reciprocal(rstack1[:], lstack1[:])
        r1 = []
        for j in range(NSLOT):
            r1.append(attn_finish(j, av1[j], rstack1, C["cwo"], None, "r1"))
        if debug:
            nc.sync.dma_start(dbg["d_avn"].ap()[:], av1[0][0][:])
            nc.sync.dma_start(dbg["d_r1"].ap()[:], r1[0][0][:])

        # ================= stages 2 (row) and 3 (col) =================
        def axial_stage(pre, res, res_tag, sname):
            x_all = [res[f][tt] for f in range(NSLOT) for tt in range(2)]
            aggs, rs = ln_stats(x_all, sname)
            lst = st.tile([32, 256], F32, tag=f"lstack_{sname}")
            avs = []
            for f in range(NSLOT):
                xh = [ln_apply(res[f][tt], aggs, rs, 2 * f + tt, sname)
                      for tt in range(2)]
                xT = transpose_pair(xh, sname)
                if debug and f == 0 and pre == "r":
                    nc.sync.dma_start(dbg["d_xh"].ap()[:], xh[0][:])
                    nc.gpsimd.dma_start(dbg["d_xT"].ap()[:], xT[0][:])
                qsrc = [[(C[f"{pre}wq{ch}{g}"][:], xT[ch][:]) for ch in range(2)]
                        for g in range(2)]
                ksrc = [[(C[f"{pre}wk{ch}{g}"][:], xT[ch][:]) for ch in range(2)]
                        for g in range(2)]
                def vsrc(i, xT=xT):
                    return [(xT[ch][0:97, 128 * i:128 * i + 128],
                             C[f"{pre}wv{ch}"][:]) for ch in range(2)]
                av_sb = attn_core(f, qsrc, ksrc, vsrc,
                                  (C[f"{pre}ka"][:], C[f"{pre}qa"][:]),
                                  NQ, 27, lst, pre)
                avs.append(av_sb)
            rst = st.tile([32, 256], BF16, tag=f"rstack_{sname}")
            nc.vector.reciprocal(rst[:], lst[:])
            return [attn_finish(f, avs[f], rst, C[f"{pre}wo"], res[f], res_tag)
                    for f in range(NSLOT)]

        def dump_partial(res):
            for f in range(NSLOT):
                for tt in range(2):
                    row = 128 * (2 * f + tt)
                    nc.gpsimd.dma_start(out.ap()[row:row + 128, 0:D], res[f][tt][:])

        if stop_stage == 1:
            dump_partial(r1)
            r2 = None
        else:
            r2 = axial_stage("r", r1, "r2", "s2")
        if debug and r2 is not None:
            nc.sync.dma_start(dbg["d_r2"].ap()[:], r2[0][0][:])
        if stop_stage == 2 and r2 is not None:
            dump_partial(r2)
        r3 = axial_stage("l", r2, "r3", "s3") if stop_stage >= 3 else None
        if debug and r3 is not None:
            nc.sync.dma_start(dbg["d_r3"].ap()[:], r3[0][0][:])

        # ================= stage 4: FFN + head =================
        if stop_stage == 3 and r3 is not None:
            dump_partial(r3)
        x_all = [r3[f][tt] for f in range(NSLOT) for tt in range(2)] \
            if stop_stage >= 4 else []
        aggs4, rs4 = ln_stats(x_all, "s4") if stop_stage >= 4 else (None, None)
        for f in range(NSLOT if stop_stage >= 4 else 0):
            xh = [ln_apply(r3[f][tt], aggs4, rs4, 2 * f + tt, "s4")
                  for tt in range(2)]
            xT = transpose_pair(xh, "s4")
            h1g = []
            for q in range(6):
                ph = pg(128, 256)
                for ch in range(2):
                    nc.tensor.matmul(ph[:],
                                     C["fw1"][ch][:, 128 * q:128 * q + 128],
                                     xT[ch][:], start=(ch == 0), stop=(ch == 1))
                hg = dyn.tile([128, 256], BF16, tag=f"h1g{q}")
                nc.scalar.activation(hg[:], ph[:], gelu_f)
                h1g.append(hg)
            if debug and f == 0:
                nc.gpsimd.dma_start(dbg["d_h1g"].ap()[:], h1g[0][:])
            z = []
            for tt in range(2):
                pz = pg(128, D)
                for q in range(6):
                    nc.tensor.matmul(pz[:], h1g[q][:, 128 * tt:128 * tt + 128],
                                     C["fw2"][:, D * q:D * q + D],
                                     start=(q == 0), stop=(q == 5))
                zt = dy3.tile([128, D], F32, tag="z_t")
                nc.vector.tensor_tensor(zt[:], pz[:], C["fb2"][:], OP.add)
                z_sb = st.tile([128, D], BF16, tag=f"z_{f}_{tt}")
                nc.vector.tensor_tensor(z_sb[:], zt[:], r3[f][tt][:], OP.add)
                z.append(z_sb)
            if debug and f == 0:
                nc.sync.dma_start(dbg["d_z"].ap()[:], z[0][:])
            zT = transpose_pair(z, "hz", ones_row=False)
            for tt in range(2):
                po = pg(128, QL)
                for ch in range(2):
                    nc.tensor.matmul(po[:], zT[ch][0:96, 128 * tt:128 * tt + 128],
                                     C["hw"][:, QL * ch:QL * ch + QL],
                                     start=(ch == 0), stop=(ch == 1))
                ot = dy3.tile([128, QL], F32, tag="o_t")
                nc.vector.tensor_tensor(ot[:], po[:], C["hb"][:], OP.add)
                row = 128 * (2 * f + tt)
                nc.gpsimd.dma_start(out.ap()[row:row + 128, :], ot[:])

    nc.compile()
    _CACHE[key] = nc
    return nc


# ---------------------------------------------------------------- entry

def kernel(**inputs):
    inputs = {k: np.asarray(v, np.float32) for k, v in inputs.items()}
    nc = build_program()
    const = _host_constants(inputs)
    in_maps = [_core_inputs(inputs, const, c) for c in range(NCORE)]
    res = bass_utils.run_bass_kernel_spmd(nc, in_maps, core_ids=list(range(NCORE)))
    out = np.zeros((B, T - 1, H, W, QL), np.float32)
    for f in range(30):
        b, t = _frame(f)
        core, j = f % 8, f // 8
        o = res.results[core]["out"].reshape(NSLOT, 2 * 128, QL)
        out[b, t - 1] = o[j].reshape(H, W, QL)
    return out



# revision 45
# speedup vs baseline: 2.9120x; 1.0797x over previous
"""Trainium2 Bass kernel for nn_DecoderVectorized (axial decoder with causal
cross-attention). Self-contained: hardcodes all shapes/sharding.

Sharding: 32 SPMD slots = 8 cores x 4 slots over the B*(T-1)=30 frames
(sorted by t so slot j has kv prefix 128*{4,8,12,15}; 2 dummy slots).
"""
import math
import sys

import numpy as np

try:
    import concourse.bass as bass
except ImportError:  # pragma: no cover
    sys.path.insert(0, "/opt/trn_rl_repo")
    import concourse.bass as bass

import concourse.bacc as bacc
import concourse.mybir as mybir
import concourse.tile as tile
from concourse import bass_utils
from concourse.masks import make_identity

F32 = mybir.dt.float32
F32R = mybir.dt.float32r
BF16 = mybir.dt.bfloat16
import ml_dtypes
NPBF = ml_dtypes.bfloat16
AF = mybir.ActivationFunctionType
OP = mybir.AluOpType

H, W, D, HEADS, QL = 16, 16, 192, 8, 256
B, T, M = 2, 16, 128
NQ = H * W          # 256 tokens per frame
DH = D // HEADS     # 24
NCORE, NSLOT = 8, 4
TMAX = [4, 8, 12, 16]
KV = [t * 128 for t in TMAX]        # 512 1024 1536 2048
CM = 32.0                           # mask bias (power of 2: bf16-exact)
SCL = 1.0 / math.sqrt(DH)
EPS = 1e-5


def _frame(f):
    """frame index f in [0,32) -> (b, t); 30/31 are dummies."""
    if f >= 30:
        return (f - 30, 15)
    return (f % 2, f // 2 + 1)


def _layouts():
    """Column layouts for the two packed const tensors (few big DMAs beat
    ~45 small ones: HWDGE is serial at ~625ns each).
    Entries: (piece_key, src_name, src_row0, rows, cols)."""
    l1, l2 = [], []
    for gg in range(2):
        l1 += [(f"cwq{gg}a", f"cwq{gg}", 0, 96, 128),
               (f"cwq{gg}b", f"cwq{gg}", 96, 97, 128),
               (f"cwk{gg}a", f"cwk{gg}", 0, 96, 128),
               (f"cwk{gg}b", f"cwk{gg}", 96, 97, 128)]
    l1 += [("cwva", "cwv", 0, 96, 256), ("cwvb", "cwv", 96, 97, 256),
           ("cwo", "cwo", 0, 128, 384)]
    for pre in ("r", "l"):
        for ch in range(2):
            for gg in range(2):
                l2 += [(f"{pre}wq{ch}{gg}", f"{pre}wq{ch}{gg}", 0, 97, 128),
                       (f"{pre}wk{ch}{gg}", f"{pre}wk{ch}{gg}", 0, 97, 128)]
            l2 += [(f"{pre}wv{ch}", f"{pre}wv{ch}", 0, 97, 256)]
        l2 += [(f"{pre}wo", f"{pre}wo", 0, 128, 384),
               (f"{pre}ka", f"{pre}ka", 0, 128, 256),
               (f"{pre}qa", f"{pre}qa", 0, 128, 256)]
    l2 += [("fw1a", "fw1", 0, 97, 768), ("fw1b", "fw1", 97, 97, 768),
           ("fw2", "fw2", 0, 128, 1152), ("hw", "hw", 0, 96, 512),
           ("repl", "repl", 0, 32, 1024)]
    return l1, l2


_LAY1, _LAY2 = _layouts()
_W1 = sum(e[4] for e in _LAY1)
_W2 = sum(e[4] for e in _LAY2)


# ---------------------------------------------------------------- host prep

def _qk_colmat_s1(w, bvec, g, is_q):
    """[193,128] colmat for stage-1 qhT/khT half g. Head h data at cols
    32h+1..32h+24 (aux row 0 first); col 32h+0: e_192 (ones) if is_q."""
    m = np.zeros((193, 128), np.float32)
    for h in range(4):
        Hh = 4 * g + h
        m[0:192, 32 * h + 1:32 * h + 25] = w[:, DH * Hh:DH * Hh + DH]
        m[192, 32 * h + 1:32 * h + 25] = bvec[DH * Hh:DH * Hh + DH]
        if is_q:
            m[192, 32 * h] = 1.0
    return m


def _qk_colmat_23(w, bvec, ch, g):
    """[97,128] colmat chunk for stage-2/3 qhT/khT. Data at cols 32h+0..+23
    (32-aligned partition start for the data matmul); axial-mask aux rows
    are contracted from separate const tiles."""
    m = np.zeros((97, 128), np.float32)
    for h in range(4):
        Hh = 4 * g + h
        m[0:96, 32 * h:32 * h + 24] = w[96 * ch:96 * ch + 96, DH * Hh:DH * Hh + DH]
        m[96, 32 * h:32 * h + 24] = bvec[DH * Hh:DH * Hh + DH] * 0.5
    return m


def _wv_colmat_s1(w, bvec):
    """[193,256]: head Hh data at cols 32Hh..+23, ones col at 32Hh+24."""
    m = np.zeros((193, 256), np.float32)
    for Hh in range(8):
        m[0:192, 32 * Hh:32 * Hh + 24] = w[:, DH * Hh:DH * Hh + DH]
        m[192, 32 * Hh:32 * Hh + 24] = bvec[DH * Hh:DH * Hh + DH]
        m[192, 32 * Hh + 24] = 1.0
    return m


def _wv_colmat_23(w, bvec, ch):
    m = np.zeros((97, 256), np.float32)
    for Hh in range(8):
        m[0:96, 32 * Hh:32 * Hh + 24] = w[96 * ch:96 * ch + 96, DH * Hh:DH * Hh + DH]
        m[96, 32 * Hh:32 * Hh + 24] = bvec[DH * Hh:DH * Hh + DH] * 0.5
        m[96, 32 * Hh + 24] = 0.5
    return m


def _wo_aug(w, bvec):
    """[128, 384]: head H=4g+h at partition rows 32h..32h+31, col block 192g:
    rows 0..23 = wo rows, row 24 = bo/8 (walrus needs lhsT/rhs same base)."""
    m = np.zeros((128, 2 * D), np.float32)
    for g in range(2):
        for h in range(4):
            Hh = 4 * g + h
            m[32 * h:32 * h + 24, D * g:D * g + D] = w[DH * Hh:DH * Hh + DH, :]
            m[32 * h + 24, D * g:D * g + D] = bvec / 8.0
    return m


def _aux_rows(idx, is_q):
    """[128,256] aux contraction rows for the rank-3 axial mask, pre-spread to
    partition rows 32h+0..2. k-side: [ri^2, ri, 1]; q-side: [-c, 2c rj, -c rj^2]."""
    r = idx.astype(np.float32)
    if is_q:
        rows = np.stack([np.full(NQ, -CM, np.float32), 2.0 * CM * r, -CM * r * r])
    else:
        rows = np.stack([r * r, r, np.ones(NQ, np.float32)])
    m = np.zeros((128, NQ), np.float32)
    for h in range(4):
        m[32 * h:32 * h + 3] = rows
    return m


def _host_constants(inp):
    """Shared (core-independent) device constant arrays."""
    c = {}
    g, b_ = inp["rn_g"], inp["rn_b"]

    def eff(wq, bq, scale):
        return (g[:, None] * wq * scale).astype(np.float32), \
               ((b_ @ wq + bq) * scale).astype(np.float32)

    for gg in range(2):
        c[f"cwq{gg}"] = _qk_colmat_s1(inp["c_wq"] * SCL, inp["c_bq"] * SCL, gg, True)
        c[f"cwk{gg}"] = _qk_colmat_s1(inp["c_wk"], inp["c_bk"], gg, False)
    c["cwv"] = _wv_colmat_s1(inp["c_wv"], inp["c_bv"])
    c["cwo"] = _wo_aug(inp["c_wo"], inp["c_bo"])
    tok = np.arange(NQ)
    for pre, wp, idx in (("r", "r", tok // 16), ("l", "col", tok % 16)):
        wq, bq = eff(inp[wp + "_wq"], inp[wp + "_bq"], SCL)
        wk, bk = eff(inp[wp + "_wk"], inp[wp + "_bk"], 1.0)
        wv, bv = eff(inp[wp + "_wv"], inp[wp + "_bv"], 1.0)
        for ch in range(2):
            for gg in range(2):
                c[f"{pre}wq{ch}{gg}"] = _qk_colmat_23(wq, bq, ch, gg)
                c[f"{pre}wk{ch}{gg}"] = _qk_colmat_23(wk, bk, ch, gg)
            c[f"{pre}wv{ch}"] = _wv_colmat_23(wv, bv, ch)
        c[f"{pre}wo"] = _wo_aug(inp[wp + "_wo"], inp[wp + "_bo"])
        c[f"{pre}ka"] = _aux_rows(idx, False)
        c[f"{pre}qa"] = _aux_rows(idx, True)
    w1 = (inp["ff_ln_g"][:, None] * inp["ff_w1"]).astype(np.float32)
    b1 = (inp["ff_ln_b"] @ inp["ff_w1"] + inp["ff_b1"]).astype(np.float32)
    fw1 = np.zeros((2 * 97, 4 * D), np.float32)
    for ch in range(2):
        fw1[97 * ch:97 * ch + 96] = w1[96 * ch:96 * ch + 96]
        fw1[97 * ch + 96] = b1 * 0.5
    c["fw1"] = fw1
    fw2 = np.zeros((128, 6 * D), np.float32)
    for q in range(6):
        fw2[:, D * q:D * q + D] = inp["ff_w2"][128 * q:128 * q + 128, :]
    c["fw2"] = fw2
    c["fb2"] = np.broadcast_to(inp["ff_b2"][None], (128, D)).copy().astype(np.float32)
    hw = np.zeros((96, 2 * QL), np.float32)
    hw[:, 0:QL] = inp["head_w"][0:96]
    hw[:, QL:2 * QL] = inp["head_w"][96:192]
    c["hw"] = hw
    c["hb"] = np.broadcast_to(inp["head_b"][None], (128, QL)).copy().astype(np.float32)
    rp = np.zeros((32, 8 * 128), np.float32)
    for k in range(8):
        for h in range(4):
            rp[4 * k + h, 128 * k + 32 * h:128 * k + 32 * h + 32] = 1.0
    c["repl"] = rp
    # pack everything into 2 bf16 + 1 f32 DMA payloads
    out = {}
    for nm, lay, W in (("cp1", _LAY1, _W1), ("cp2", _LAY2, _W2)):
        arr = np.zeros((128, W), NPBF)
        off = 0
        for key, src, r0, rows, cols in lay:
            arr[0:rows, off:off + cols] = c[src][r0:r0 + rows].astype(NPBF)
            off += cols
        out[nm] = arr
    f = np.zeros((128, 448), np.float32)
    f[:, 0:192] = c["fb2"]
    f[:, 192:448] = c["hb"]
    out["cpf"] = f
    return out


def _core_inputs(inp, const, core):
    """Per-core in_map (includes the shared consts)."""
    m = dict(const)
    qg = np.asarray(inp["query_grid"], np.float32)
    tp = np.asarray(inp["t_pos_w"], np.float32)
    mt = np.asarray(inp["mem_tokens"], np.float32)
    qT = np.zeros((NSLOT * 193, NQ), np.float32)
    for j in range(NSLOT):
        b, t = _frame(8 * j + core)
        qT[193 * j:193 * j + 192] = (qg + tp[t][None, :]).T
        qT[193 * j + 192] = 1.0
        kvT = np.ones((193, KV[j]), np.float32)
        kvT[0:192] = mt[b, :TMAX[j]].reshape(-1, D).T
        m[f"kvT{j}"] = kvT.astype(NPBF)
        msk = np.zeros((4, KV[j]), np.float32)
        msk[:, 128 * t:] = -CM
        m[f"msk{j}"] = msk.astype(NPBF)
    m["qT"] = qT.astype(NPBF)
    return m


# ---------------------------------------------------------------- program

_CACHE = {}


def build_program(gelu_f=AF.Gelu, debug=False, stop_stage=4):
    key = (gelu_f, debug, stop_stage)
    if key in _CACHE:
        return _CACHE[key]
    nc = bacc.Bacc("TRN2", target_bir_lowering=False, debug=False)

    # ---- DRAM I/O ----
    dr = {}
    def din(name, shape, dt_=BF16):
        dr[name] = nc.dram_tensor(name, shape, dt_, kind="ExternalInput")
    din("cp1", (128, _W1)); din("cp2", (128, _W2)); din("cpf", (128, 448), F32)
    din("qT", (NSLOT * 193, NQ))
    for j in range(NSLOT):
        din(f"kvT{j}", (193, KV[j])); din(f"msk{j}", (4, KV[j]))
    out = nc.dram_tensor("out", (NSLOT * 2 * 128, QL), F32, kind="ExternalOutput")
    dbg = {}
    if debug:
        for nm, shape, dt_ in (("d_qhT", (128, 256), BF16), ("d_khT", (128, 512), BF16),
                               ("d_pT", (128, 1024), BF16), ("d_av", (128, 256), F32),
                               ("d_avn", (128, 256), F32), ("d_r1", (128, D), F32),
                               ("d_xh", (128, D), BF16), ("d_xT", (97, 256), BF16),
                               ("d_r2", (128, D), F32), ("d_r3", (128, D), F32),
                               ("d_h1g", (128, 256), BF16), ("d_z", (128, D), BF16)):
            dbg[nm] = nc.dram_tensor(nm, shape, dt_, kind="ExternalOutput")

    from contextlib import ExitStack
    with tile.TileContext(nc) as tc, ExitStack() as es, \
            nc.allow_low_precision(reason="bf16 operands; rel-err gate 2e-2"):
        cst = es.enter_context(tc.tile_pool(name="cst", bufs=1))
        dyn = es.enter_context(tc.tile_pool(name="dyn", bufs=2))
        dy3 = es.enter_context(tc.tile_pool(name="dy3", bufs=3))
        dy8 = es.enter_context(tc.tile_pool(name="dy8", bufs=8))
        st = es.enter_context(tc.tile_pool(name="st", bufs=1))
        ps_s = es.enter_context(tc.tile_pool(name="ps_s", bufs=2, space="PSUM"))
        ps_a = es.enter_context(tc.tile_pool(name="ps_a", bufs=2, space="PSUM"))
        ps_g = es.enter_context(tc.tile_pool(name="ps_g", bufs=2, space="PSUM"))

        def rr(ap):
            return ap.bitcast(mybir.dt.float32r)

        def pg(p_, f_, dt_=F32):
            return ps_g.tile([p_, f_], dt_, tag="pg", name="pg")

        # ---- slot-0 stage-1 inputs first (HWDGE is a serial queue) ----
        kva0 = dyn.tile([96, KV[0]], BF16, tag="kvTa", bufs=2)
        kvb0 = dyn.tile([97, KV[0]], BF16, tag="kvTb", bufs=2)
        nc.sync.dma_start(kva0[:], dr["kvT0"].ap()[0:96, :])
        nc.sync.dma_start(kvb0[:], dr["kvT0"].ap()[96:193, :])
        qta0 = dyn.tile([96, 256], BF16, tag="qTa")
        qtb0 = dyn.tile([97, 256], BF16, tag="qTb")
        nc.sync.dma_start(qta0[:], dr["qT"].ap()[0:96, :])
        nc.sync.dma_start(qtb0[:], dr["qT"].ap()[96:193, :])
        # ---- packed constants: 3 DMAs ----
        cp1 = cst.tile([128, _W1], BF16, tag="cp1")
        nc.sync.dma_start(cp1[:], dr["cp1"].ap()[:])
        cp2 = cst.tile([128, _W2], BF16, tag="cp2")
        nc.sync.dma_start(cp2[:], dr["cp2"].ap()[:])
        cpf = cst.tile([128, 448], F32, tag="cpf")
        nc.sync.dma_start(cpf[:], dr["cpf"].ap()[:])
        C = {}
        for tile_, lay in ((cp1, _LAY1), (cp2, _LAY2)):
            off = 0
            for key, src, r0, rows, cols in lay:
                C[key] = tile_[0:rows, off:off + cols]
                off += cols
        for gg in range(2):
            C[f"cwq{gg}"] = (C[f"cwq{gg}a"], C[f"cwq{gg}b"])
            C[f"cwk{gg}"] = (C[f"cwk{gg}a"], C[f"cwk{gg}b"])
        C["cwv"] = (C["cwva"], C["cwvb"])
        C["fw1"] = (C["fw1a"], C["fw1b"])
        C["fb2"] = cpf[0:128, 0:192]
        C["hb"] = cpf[0:128, 192:448]
        ident = cst.tile([128, 128], BF16, tag="ident")
        make_identity(nc, ident[:])
        epsc = cst.tile([128, 1], F32, tag="epsc")
        nc.gpsimd.memset(epsc[:], EPS)

        def evac(dst_ap, src_ap):
            nc.vector.tensor_copy(dst_ap, src_ap)

        # ============ generic attention core ============
        def attn_core(iid, qsrc, ksrc, vsrc, kaux, kvlen, kp, lstack, pres):
            """qsrc/ksrc: per half, list of (lhsT_chunk, rhs_chunk) pairs.
            vsrc(i): chunk pairs for kv-tile i. kaux: s1 -> DRAM msk ap
            (1 bias row/head, strided-DMA'd into khT rows 32h); s2/3 ->
            (ka, qa) const tiles contracted via a separate K=3 matmul.
            Returns av_sb (unnormalized ovT + l rows) per half."""
            nkt = kvlen // 128
            qhT, khT = [], []
            for g in range(2):
                pq = pg(128, 256)
                for i, (cm, rhs) in enumerate(qsrc[g]):
                    nc.tensor.matmul(pq[:], cm, rhs, start=(i == 0),
                                     stop=(i == len(qsrc[g]) - 1))
                q_sb = dyn.tile([128, 256], BF16, tag=f"qhT{g}")
                evac(q_sb[:], pq[:])
                qhT.append(q_sb)
                k_sb = dyn.tile([128, kvlen], BF16, tag=f"khT{g}", bufs=1)
                for nchunk in range(0, kvlen, 512):
                    nw = min(512, kvlen - nchunk)
                    pk = pg(128, 512)
                    for i, (cm, rhs) in enumerate(ksrc[g]):
                        nc.tensor.matmul(pk[:, 0:nw], cm,
                                         rhs[:, nchunk:nchunk + nw],
                                         start=(i == 0),
                                         stop=(i == len(ksrc[g]) - 1))
                    evac(k_sb[:, nchunk:nchunk + nw], pk[:, 0:nw])
                if pres == "s1":          # dynamic mask row -> partitions 32h
                    nc.gpsimd.dma_start(k_sb[0:97:32, :], kaux[:])
                khT.append(k_sb)
            av = [ps_a.tile([128, 256], F32, tag="p_av", name="p_av") for _ in range(2)]
            npair = nkt // 2

            def emit_av(ip, g, vhs, pTs):
                for hp in range(2):
                    for hh in range(2):
                        h = 2 * hp + hh
                        for d in range(2):
                            nc.tensor.matmul(
                                av[g][32 * h:32 * h + 32, :],
                                vhs[d][:, 32 * (4 * g + h):32 * (4 * g + h) + 32],
                                pTs[hp][:, 512 * hh + 256 * d:
                                        512 * hh + 256 * d + 256],
                                start=(ip == 0 and d == 0),
                                stop=(ip == npair - 1 and d == 1),
                                tile_position=(0, 32 * h), skip_group_check=True)

            pend = None   # AV mms run one (ip,g)-unit late so the PE stream
            vhs = None    # always has exp-independent work in front of them
            for ip in range(npair):
                for g in range(2):
                    if g == 0:
                        vhs = []
                        for d in range(2):
                            pv = pg(128, 256)
                            vch = vsrc(2 * ip + d)
                            for ci, (cm, rhs) in enumerate(vch):
                                nc.tensor.matmul(pv[:], cm, rhs, start=(ci == 0),
                                                 stop=(ci == len(vch) - 1))
                            vh = dy3.tile([128, 256], BF16, tag="vh", bufs=4)
                            evac(vh[:], pv[:])
                            vhs.append(vh)
                    # two head-pair psum tiles (bufs=2): PE fills one while
                    # Act exps the other. One head per psum bank (cols 512hh)
                    # -- concurrent row-group matmuls to one bank collide
                    pTs = []
                    for hp in range(2):
                        psd = ps_s.tile([128, 1024], F32, tag="p_sT", bufs=2,
                                        name="p_sT")
                        for hh in range(2):
                            h = 2 * hp + hh
                            for d in range(2):
                                sl = psd[:, 512 * hh + 256 * d:
                                         512 * hh + 256 * d + 256]
                                ck = 128 * (2 * ip + d)
                                if pres == "s1":
                                    nc.tensor.matmul(
                                        sl, khT[g][32 * h:32 * h + kp,
                                                   ck:ck + 128],
                                        qhT[g][32 * h:32 * h + kp, :],
                                        start=True, stop=True,
                                        tile_position=(32 * h, 0))
                                else:
                                    nc.tensor.matmul(
                                        sl, khT[g][32 * h:32 * h + 24,
                                                   ck:ck + 128],
                                        qhT[g][32 * h:32 * h + 24, :],
                                        start=True, stop=False,
                                        tile_position=(32 * h, 0))
                                    nc.tensor.matmul(
                                        sl, kaux[0][32 * h:32 * h + 3,
                                                    ck:ck + 128],
                                        kaux[1][32 * h:32 * h + 3, :],
                                        start=False, stop=True,
                                        tile_position=(32 * h, 0))
                        pTd = dy3.tile([128, 1024], BF16, tag="pT", bufs=4)
                        nc.scalar.activation(pTd[:], psd[:], AF.Exp)
                        pTs.append(pTd)
                    if debug and iid == 0 and ip == 0 and g == 0 and pres == "s1":
                        nc.gpsimd.dma_start(dbg["d_pT"].ap()[:], pTs[0][:])
                    if pend is not None:
                        emit_av(*pend)
                    pend = (ip, g, list(vhs), pTs)
            emit_av(*pend)
            av_sb = []
            for g in range(2):
                a_sb = dy8.tile([128, 256], F32, tag="av_sb")
                evac(a_sb[:], av[g][:])
                # l rows live at partitions 32h+24: strided DMA extracts all 4
                nc.gpsimd.dma_start(lstack[8 * iid + 4 * g:8 * iid + 4 * g + 4, :],
                                    a_sb[24:121:32, :])
                av_sb.append(a_sb)
            if debug and iid == 0 and pres == "s1":
                nc.gpsimd.dma_start(dbg["d_qhT"].ap()[:], qhT[0][:])
                nc.gpsimd.dma_start(dbg["d_khT"].ap()[:], khT[0][:, 0:512])
                nc.sync.dma_start(dbg["d_av"].ap()[:], av_sb[0][:])
            return av_sb

        def attn_finish(iid, av_sb, rstack, wo_t, res_in, res_tag):
            """r-broadcast, normalize, wo projection (+residual)."""
            avn = []
            for g in range(2):
                prb = pg(128, 256)
                k = 2 * iid + g
                nc.tensor.matmul(prb[:], C["repl"][:, 128 * k:128 * k + 128],
                                 rstack[:], start=True, stop=True)
                an = dy3.tile([128, 256], BF16, tag="avn")
                nc.vector.tensor_tensor(an[:], av_sb[g][:], prb[:], OP.mult)
                avn.append(an)
            outs = []
            for tt in range(2):
                py = pg(128, D)
                for g in range(2):
                    nc.tensor.matmul(
                        py[:], avn[g][:, 128 * tt:128 * tt + 128],
                        wo_t[:, D * g:D * g + D],
                        start=(g == 0), stop=(g == 1))
                r_new = st.tile([128, D], F32, tag=f"{res_tag}_{iid}_{tt}")
                if res_in is None:
                    evac(r_new[:], py[:])
                else:
                    nc.vector.tensor_tensor(r_new[:], res_in[tt][:], py[:], OP.add)
                outs.append(r_new)
            return outs

        # ============ LN helpers ============
        def ln_stats(x_tiles, name):
            # tensor_tensor_reduce is fatal at runtime on this stack; use
            # bn_stats/bn_aggr (one DVE pass -> mean, var per partition)
            n = len(x_tiles)
            var = dyn.tile([128, n], F32, tag=f"var_{name}")
            rs = dyn.tile([128, n], F32, tag=f"rs_{name}")
            aggs = []
            for k, xt in enumerate(x_tiles):
                bst = dy3.tile([128, 6], F32, tag="bst")
                nc.vector.bn_stats(bst[:], xt[:])
                agg = dy8.tile([128, 2], F32, tag=f"agg_{name}", name="agg")
                nc.vector.bn_aggr(agg[:], bst[:])
                nc.vector.tensor_copy(var[:, k:k + 1], agg[:, 1:2])
                aggs.append(agg)
            lnv = dyn.tile([128, n], F32, tag=f"lnv_{name}")
            nc.scalar.activation(lnv[:], var[:], AF.Ln, bias=epsc[:])
            nc.scalar.activation(rs[:], lnv[:], AF.Exp, scale=-0.5)
            return aggs, rs

        def ln_apply(x, aggs, rs, k, name):
            xh = dy3.tile([128, D], BF16, tag=f"xh_{name}")
            nc.vector.tensor_scalar(xh[:], x[:], aggs[k][:, 0:1], rs[:, k:k + 1],
                                    OP.subtract, OP.mult)
            return xh

        def transpose_pair(xh_tiles, name, ones_row=True):
            xT = []
            for ch in range(2):
                t_ = dyn.tile([97, 256], BF16, tag=f"xT{ch}", name=f"xT{ch}")
                for tt in range(2):
                    pt = pg(96, 128, BF16)
                    nc.tensor.transpose(pt[:], xh_tiles[tt][:, 96 * ch:96 * ch + 96],
                                        ident[:])
                    evac(t_[0:96, 128 * tt:128 * tt + 128], pt[:])
                if ones_row:
                    nc.gpsimd.memset(t_[96:97, :], 1.0)
                xT.append(t_)
            return xT

        # ================= stage 1: cross attention =================
        lstack1 = st.tile([32, 256], F32, tag="lstack1")
        av1 = []
        for j in range(NSLOT):
            if j == 0:
                kva, kvb, qta, qtb = kva0, kvb0, qta0, qtb0
            else:
                kva = dyn.tile([96, KV[j]], BF16, tag="kvTa", bufs=2)
                kvb = dyn.tile([97, KV[j]], BF16, tag="kvTb", bufs=2)
                nc.sync.dma_start(kva[:], dr[f"kvT{j}"].ap()[0:96, :])
                nc.sync.dma_start(kvb[:], dr[f"kvT{j}"].ap()[96:193, :])
                qta = dyn.tile([96, 256], BF16, tag="qTa")
                qtb = dyn.tile([97, 256], BF16, tag="qTb")
                nc.sync.dma_start(qta[:], dr["qT"].ap()[193 * j:193 * j + 96, :])
                nc.sync.dma_start(qtb[:],
                                  dr["qT"].ap()[193 * j + 96:193 * j + 193, :])
            qsrc = [[(C[f"cwq{g}"][0][:], qta[:]), (C[f"cwq{g}"][1][:], qtb[:])]
                    for g in range(2)]
            ksrc = [[(C[f"cwk{g}"][0][:], kva[:]), (C[f"cwk{g}"][1][:], kvb[:])]
                    for g in range(2)]
            def vsrc(i, kva=kva, kvb=kvb):
                return [(kva[:, 128 * i:128 * i + 128], C["cwv"][0][:]),
                        (kvb[:, 128 * i:128 * i + 128], C["cwv"][1][:])]
            av_sb = attn_core(j, qsrc, ksrc, vsrc, dr[f"msk{j}"].ap()[:],
                              KV[j], 25, lstack1, "s1")
            av1.append(av_sb)
        rstack1 = st.tile([32, 256], BF16, tag="rstack1")
        nc.vector.reciprocal(rstack1[:], lstack1[:])
        r1 = []
        for j in range(NSLOT):
            r1.append(attn_finish(j, av1[j], rstack1, C["cwo"], None, "r1"))
        if debug:
            nc.sync.dma_start(dbg["d_avn"].ap()[:], av1[0][0][:])
            nc.sync.dma_start(dbg["d_r1"].ap()[:], r1[0][0][:])

        # ================= stages 2 (row) and 3 (col) =================
        def axial_stage(pre, res, res_tag, sname):
            x_all = [res[f][tt] for f in range(NSLOT) for tt in range(2)]
            aggs, rs = ln_stats(x_all, sname)
            lst = st.tile([32, 256], F32, tag=f"lstack_{sname}")
            avs = []
            for f in range(NSLOT):
                xh = [ln_apply(res[f][tt], aggs, rs, 2 * f + tt, sname)
                      for tt in range(2)]
                xT = transpose_pair(xh, sname)
                if debug and f == 0 and pre == "r":
                    nc.sync.dma_start(dbg["d_xh"].ap()[:], xh[0][:])
                    nc.gpsimd.dma_start(dbg["d_xT"].ap()[:], xT[0][:])
                qsrc = [[(C[f"{pre}wq{ch}{g}"][:], xT[ch][:]) for ch in range(2)]
                        for g in range(2)]
                ksrc = [[(C[f"{pre}wk{ch}{g}"][:], xT[ch][:]) for ch in range(2)]
                        for g in range(2)]
                def vsrc(i, xT=xT):
                    return [(xT[ch][0:97, 128 * i:128 * i + 128],
                             C[f"{pre}wv{ch}"][:]) for ch in range(2)]
                av_sb = attn_core(f, qsrc, ksrc, vsrc,
                                  (C[f"{pre}ka"][:], C[f"{pre}qa"][:]),
                                  NQ, 27, lst, pre)
                avs.append(av_sb)
            rst = st.tile([32, 256], BF16, tag=f"rstack_{sname}")
            nc.vector.reciprocal(rst[:], lst[:])
            return [attn_finish(f, avs[f], rst, C[f"{pre}wo"], res[f], res_tag)
                    for f in range(NSLOT)]

        def dump_partial(res):
            for f in range(NSLOT):
                for tt in range(2):
                    row = 128 * (2 * f + tt)
                    nc.gpsimd.dma_start(out.ap()[row:row + 128, 0:D], res[f][tt][:])

        if stop_stage == 1:
            dump_partial(r1)
            r2 = None
        else:
            r2 = axial_stage("r", r1, "r2", "s2")
        if debug and r2 is not None:
            nc.sync.dma_start(dbg["d_r2"].ap()[:], r2[0][0][:])
        if stop_stage == 2 and r2 is not None:
            dump_partial(r2)
        r3 = axial_stage("l", r2, "r3", "s3") if stop_stage >= 3 else None
        if debug and r3 is not None:
            nc.sync.dma_start(dbg["d_r3"].ap()[:], r3[0][0][:])

        # ================= stage 4: FFN + head =================
        if stop_stage == 3 and r3 is not None:
            dump_partial(r3)
        x_all = [r3[f][tt] for f in range(NSLOT) for tt in range(2)] \
            if stop_stage >= 4 else []
        aggs4, rs4 = ln_stats(x_all, "s4") if stop_stage >= 4 else (None, None)
        for f in range(NSLOT if stop_stage >= 4 else 0):
            xh = [ln_apply(r3[f][tt], aggs4, rs4, 2 * f + tt, "s4")
                  for tt in range(2)]
            xT = transpose_pair(xh, "s4")
            h1g = []
            for q in range(6):
                ph = pg(128, 256)
                for ch in range(2):
                    nc.tensor.matmul(ph[:],
                                     C["fw1"][ch][:, 128 * q:128 * q + 128],
                                     xT[ch][:], start=(ch == 0), stop=(ch == 1))
                hg = dyn.tile([128, 256], BF16, tag=f"h1g{q}")
                nc.scalar.activation(hg[:], ph[:], gelu_f)
                h1g.append(hg)
            if debug and f == 0:
                nc.gpsimd.dma_start(dbg["d_h1g"].ap()[:], h1g[0][:])
            z = []
            for tt in range(2):
                pz = pg(128, D)
                for q in range(6):
                    nc.tensor.matmul(pz[:], h1g[q][:, 128 * tt:128 * tt + 128],
                                     C["fw2"][:, D * q:D * q + D],
                                     start=(q == 0), stop=(q == 5))
                zt = dy3.tile([128, D], F32, tag="z_t")
                nc.vector.tensor_tensor(zt[:], pz[:], C["fb2"][:], OP.add)
                z_sb = st.tile([128, D], BF16, tag=f"z_{f}_{tt}")
                nc.vector.tensor_tensor(z_sb[:], zt[:], r3[f][tt][:], OP.add)
                z.append(z_sb)
            if debug and f == 0:
                nc.sync.dma_start(dbg["d_z"].ap()[:], z[0][:])
            zT = transpose_pair(z, "hz", ones_row=False)
            for tt in range(2):
                po = pg(128, QL)
                for ch in range(2):
                    nc.tensor.matmul(po[:], zT[ch][0:96, 128 * tt:128 * tt + 128],
                                     C["hw"][:, QL * ch:QL * ch + QL],
                                     start=(ch == 0), stop=(ch == 1))
                ot = dy3.tile([128, QL], F32, tag="o_t")
                nc.vector.tensor_tensor(ot[:], po[:], C["hb"][:], OP.add)
                row = 128 * (2 * f + tt)
                nc.gpsimd.dma_start(out.ap()[row:row + 128, :], ot[:])

    nc.compile()
    _CACHE[key] = nc
    return nc


# ---------------------------------------------------------------- entry

def kernel(**inputs):
    inputs = {k: np.asarray(v, np.float32) for k, v in inputs.items()}
    nc = build_program()
    const = _host_constants(inputs)
    in_maps = [_core_inputs(inputs, const, c) for c in range(NCORE)]
    res = bass_utils.run_bass_kernel_spmd(nc, in_maps, core_ids=list(range(NCORE)))
    out = np.zeros((B, T - 1, H, W, QL), np.float32)
    for f in range(30):
        b, t = _frame(f)
        core, j = f % 8, f // 8
        o = res.results[core]["out"].reshape(NSLOT, 2 * 128, QL)
        out[b, t - 1] = o[j].reshape(H, W, QL)
    return out

